# revision 29
# baseline (speedup 1.0000x reference)
"""Distributed single-head attention + MLP block for 8 TRN2 NeuronCores.

Reference computation (per batch b):
  Q = query @ Wq^T + bq ; K = key @ Wk^T + bk
  scores = Q @ K^T / sqrt(H) ; attn = softmax(scores)
  weighted = attn @ value + value
  h1 = relu(weighted @ Wo1^T + bo1)
  out = h1 @ Wo2^T + bo2 + weighted

Sharding: B=4 batches x 2 query-row halves = 8 shards. Each core gets its
1024 query rows plus the full 2048 keys/values of its batch; attention is
dense (non-causal) so no inter-core communication is needed.

Layout: everything on-device is feature-on-partitions ("T-layout",
X^T[f, tok]) so all matmul contractions line up with zero on-device
transposes; the host pre-packs every shard into the exact [128, free]
SBUF tiling the kernel consumes. All matmuls run fp8e4 DoubleRow (2x).

Numerics tricks (all folded on host / into activation scale operands):
  - softmax shift-invariance: K bias bk drops entirely (k-independent
    terms cancel; bq is kept on the Q side).
  - Q is stored as 32x true-Q and K as true-K so both live in fp8's
    normal range (raw Q*scale has std ~0.02 - subnormal); the scores
    PSUM is then 32x the true logits and the exp activation applies
    scale=1/32.
  - fp8 weights are stored 64x and unscaled via activation scale=1/64
    (uniform(-0.036, 0.036) weights would otherwise be ~half subnormal).
  - softmax needs no max-subtraction: logits have std ~1/3.

Schedule (PE never idles between phases):
  K-proj (24 tiles over an 8-bank PSUM rotation, DVE evictions)
  Q-proj nb=0, then attention block 0 with Q-proj nb=1 tiles interleaved
  attention block 1 with MLP(0) tiles interleaved, then MLP(1).
Attention merges scores and PV per k-tile pair: 3 score MMs -> exp on
ScalarE -> 6 PV MMs one pair behind, so PSUM-evict latency is always
covered by other matmuls. The softmax denominator accumulates on DVE
(even k-tiles) and GpSimd (odd k-tiles); the rowsum matmul sums both,
reciprocal runs on [1,512] only, and the PE broadcasts it back.
Bulk prefetches (v, vT, MLP weights) issue on the Scalar HWDGE queue
during the K-proj phase (ScalarE is idle there); everything else on Sync.
"""

import contextlib

import numpy as np
import ml_dtypes

import concourse.bass as bass
import concourse.mybir as mybir
import concourse.tile as tile
from concourse.bass_utils import run_bass_kernel_spmd

dt = mybir.dt
AF = mybir.ActivationFunctionType

H = 768          # model dim
B = 4            # batch
S = 2048         # sequence length
N_CORES = 8
QCHUNK = S * B // N_CORES        # 1024 query rows per core
HT = H // 128                    # 6 feature partition-tiles
KTILES = S // 128                # 16 key partition-tiles
QB = 512                         # q-block width (= PSUM bank, fp32)
NQB = QCHUNK // QB               # 2 q-blocks per core

FP8 = dt.float8e4
NP_FP8 = dt.np(FP8)
NP_BF = ml_dtypes.bfloat16
PMODE = mybir.MatmulPerfMode.DoubleRow

WSCALE = 64.0      # fp8 weight pre-scale (host) -> activation scale 1/64
QSCALE = 32.0      # stored Q = 32x true Q -> exp scale 1/32


def build_kernel():
    nc = bass.Bass()

    qT_ext = nc.declare_dram_parameter("qT", [128, NQB * HT * QB], FP8, isOutput=False)
    kT_ext = nc.declare_dram_parameter("kT", [128, (S // QB) * HT * QB], FP8, isOutput=False)
    v_ext = nc.declare_dram_parameter("v", [128, KTILES * H], FP8, isOutput=False)
    vT_ext = nc.declare_dram_parameter("vT", [128, NQB * HT * QB], dt.float32, isOutput=False)
    w_ext = {
        name: nc.declare_dram_parameter(name, [128, HT * H], FP8, isOutput=False)
        for name in ("wqT", "wkT", "wo1T", "wo2T")
    }
    b_ext = nc.declare_dram_parameter("biases", [128, 3 * HT], dt.float32,
                                      isOutput=False)
    id64_ext = nc.declare_dram_parameter("ident64", [128, 128], dt.bfloat16,
                                         isOutput=False)
    outT_ext = nc.declare_dram_parameter(
        "outT", [128, NQB * HT * QB], dt.float32, isOutput=True
    )

    with tile.TileContext(nc) as tc, nc.allow_low_precision(
        reason="fp8 matmul path is intentional; rel-err budget is 2e-2"
    ):
        _body(nc, tc, qT_ext, kT_ext, v_ext, vT_ext, w_ext, b_ext, id64_ext,
              outT_ext)

    _split_multi_waits(nc)
    return nc


def _body(nc, tc, qT_ext, kT_ext, v_ext, vT_ext, w_ext, b_ext, id64_ext,
          outT_ext):
    with contextlib.ExitStack() as ctx:
        const_pool = ctx.enter_context(tc.tile_pool(name="const", bufs=1))
        w_pool = ctx.enter_context(tc.tile_pool(name="w", bufs=1))
        act_pool = ctx.enter_context(tc.tile_pool(name="act", bufs=1))
        in_pool = ctx.enter_context(tc.tile_pool(name="inp", bufs=2))
        vt_pool = ctx.enter_context(tc.tile_pool(name="vt", bufs=2))
        wt_pool = ctx.enter_context(tc.tile_pool(name="wt", bufs=2))
        h1_pool = ctx.enter_context(tc.tile_pool(name="h1", bufs=2))
        st_pool = ctx.enter_context(tc.tile_pool(name="st", bufs=2))
        tmp_pool = ctx.enter_context(tc.tile_pool(name="tmp", bufs=4))
        o_pool = ctx.enter_context(tc.tile_pool(name="o", bufs=6))
        exp_pool = ctx.enter_context(tc.tile_pool(name="exps", bufs=6))
        # PSUM: 6 PV accumulators + 2 general banks = 8 banks exactly.
        ps_pool = ctx.enter_context(tc.tile_pool(name="ps", bufs=1, space="PSUM"))
        ps_gen = ctx.enter_context(tc.tile_pool(name="ps_gen", bufs=2, space="PSUM"))

        # ---- weight / bias loading helpers ----
        w_sb = {}

        def load_weight_chunk(name, j, eng=None):
            ts = w_sb.get(name)
            if ts is None:
                ts = [w_pool.tile([128, HT * 128], FP8, tag=f"{name}{g}",
                                  name=f"w_{name}{g}") for g in range(HT)]
                w_sb[name] = ts
            step = HT * 128
            (eng or nc.sync).dma_start(ts[j][:],
                                       w_ext[name][:, j * step:(j + 1) * step])

        def wpair(name, ot, j):
            """lhsT [128, 2, 128]: out-tile ot, contraction h-tile pair (2j, 2j+1)."""
            c0 = 2 * j * 128
            return (w_sb[name][ot][:, c0: c0 + 256]
                    .rearrange("p (t m) -> p t m", t=2))

        # ---- stage-1 DMAs in first-use order (Sync queue) ----
        def load_x_chunks(ext, nb, tagbase):
            """One projection input block as 3 ht-pair tiles [128, 2*QB]."""
            tiles = []
            for j in range(3):
                t = in_pool.tile([128, 2 * QB], FP8, tag=f"{tagbase}{j}",
                                 name=f"x_{tagbase}{j}_{nb}")
                c0 = nb * HT * QB + j * 2 * QB
                nc.sync.dma_start(t[:], ext[:, c0: c0 + 2 * QB])
                tiles.append(t)
            return tiles

        load_weight_chunk("wkT", 0)
        kx0 = []
        for j in range(3):
            t = in_pool.tile([128, 2 * QB], FP8, tag=f"kx{j}", name=f"x_kx{j}_0")
            nc.sync.dma_start(t[:], kT_ext[:, j * 2 * QB:(j + 1) * 2 * QB])
            kx0.append(t)
        for j in range(1, HT):
            load_weight_chunk("wkT", j, eng=nc.scalar)

        bias_sb = const_pool.tile([128, 3 * HT], dt.float32, tag="biases")
        nc.sync.dma_start(bias_sb[:], b_ext[:])
        id64_sb = const_pool.tile([128, 128], dt.bfloat16, tag="ident64")
        nc.scalar.dma_start(id64_sb[:], id64_ext[:])
        biases = {name: bias_sb[:, i * HT:(i + 1) * HT]
                  for i, name in enumerate(("bq", "bo1", "bo2"))}

        # ---- activation tiles ----
        KT = [act_pool.tile([128, HT * QB], FP8, tag=f"KT{nb}",
                            name=f"KT{nb}") for nb in range(4)]
        KT3 = [t[:].rearrange("p (t k) -> p t k", t=HT) for t in KT]
        QT = [[act_pool.tile([128, 2 * QB], FP8, tag=f"QT{qb}_{j}",
                             name=f"QT{qb}_{j}") for j in range(3)]
              for qb in range(NQB)]

        ones_f32 = const_pool.tile([128, 128], dt.float32, tag="ones_f32")
        nc.vector.memset(ones_f32[:], 1.0)
        ones_row = const_pool.tile([1, 128], dt.float32r, tag="ones_row")
        nc.vector.tensor_copy(ones_row[:], ones_f32[0:1, :])
        ones_col = const_pool.tile([128, 1], dt.float32r, tag="ones_col")
        nc.vector.tensor_copy(ones_col[:], ones_f32[:, 0:1])

        # 8-bank PSUM rotation for the projection phase (PV banks are free).
        _rot = {"i": 0}

        def proj_ps(nm):
            i = _rot["i"]
            _rot["i"] += 1
            if i % 8 < 2:
                return ps_gen.tile([128, QB], dt.float32, tag="gen", name=nm)
            return ps_pool.tile([128, QB], dt.float32, tag=f"ps_w{i % 8 - 2}",
                                name=nm)

        def proj_tile(wname, xt, ot, ps):
            """3 DoubleRow matmuls: one [128out, QB] projection tile."""
            for j in range(3):
                nc.tensor.matmul(
                    ps[:], wpair(wname, ot, j),
                    xt[j][:].rearrange("p (t q) -> p t q", t=2),
                    start=(j == 0), stop=(j == 2), perf_mode=PMODE,
                )

        # ---- K projection: 24 tiles, DVE evictions (no bias - bk drops) ----
        k_tile_fns = []

        def k_tile(nb, xt, ot):
            ps = ps_gen.tile([128, QB], dt.float32, tag="gen",
                             name=f"ps_k3_{ot}")
            proj_tile("wkT", xt, ot, ps)
            nc.vector.tensor_scalar_mul(KT[nb][:, ot * QB:(ot + 1) * QB],
                                        ps[:], 1.0 / WSCALE)

        cur = kx0
        for nb in range(4):
            nxt = load_x_chunks(kT_ext, nb + 1, "kx") if nb < 3 else None
            if nb == 3:
                # nb3 tiles are emitted inside attention block 0 (fill slots)
                kx3_saved = cur
                break
            if nb == 1:
                v_blks = []
                for c in range(4):
                    t = act_pool.tile([128, 4 * H], FP8, tag=f"v_in{c}",
                                      name=f"v_in{c}")
                    nc.scalar.dma_start(t[:], v_ext[:, c * 4 * H:(c + 1) * 4 * H])
                    v_blks.append(t)
                for j in range(HT):
                    load_weight_chunk("wo1T", j, eng=nc.scalar)
            elif nb == 2:
                for j in range(HT):
                    load_weight_chunk("wo2T", j, eng=nc.scalar)
            for ot in range(HT):
                ps = proj_ps(f"ps_k_{nb}_{ot}")
                proj_tile("wkT", cur, ot, ps)
                dst = KT[nb][:, ot * QB:(ot + 1) * QB]
                # evictions mostly on DVE; ScalarE's sequencer is busy with
                # the bulk-prefetch DIRECT2D descriptor generation early on
                # (evictions queued behind those stall the PSUM rotation),
                # but from nb>=2 it is free and can take a third of the load
                if nb >= 2 and ot % 3 == 2:
                    nc.scalar.activation(dst, ps[:], AF.Identity,
                                         scale=1.0 / WSCALE)
                else:
                    nc.vector.tensor_scalar_mul(dst, ps[:], 1.0 / WSCALE)
            cur = nxt

        def vpair(jk, ht):
            """lhsT [128, 2, 128]: k-tile pair (2jk, 2jk+1), h-tile ht."""
            t = v_blks[jk // 2]
            j2 = (jk % 2) * 2
            return (t[:].rearrange("p (t h) -> p t h", t=4)
                    [:, j2: j2 + 2, ht * 128:(ht + 1) * 128])

        # ---- Q projection nb=0 (ScalarE evictions apply bias + 1/64) ----
        for j in range(HT):
            load_weight_chunk("wqT", j)
        qx0 = load_x_chunks(qT_ext, 0, "qx")
        qx1 = load_x_chunks(qT_ext, 1, "qx")

        def q_tile(qb, xt, ot, gen_only=False):
            # interleaved tiles (inside attention) must not touch the live
            # PV accumulator banks - gen rotation only
            if gen_only:
                ps = ps_gen.tile([128, QB], dt.float32, tag="gen",
                                 name=f"ps_q_{qb}_{ot}")
            else:
                ps = proj_ps(f"ps_q_{qb}_{ot}")
            proj_tile("wqT", xt, ot, ps)
            nc.scalar.activation(
                QT[qb][ot // 2][:, (ot % 2) * QB:(ot % 2 + 1) * QB], ps[:],
                AF.Identity,
                bias=biases["bq"][:, ot: ot + 1], scale=1.0 / WSCALE,
            )

        # ---- vT (bf16 residual) as ht-pair tiles ----
        def load_vt(qb, eng):
            tiles = []
            for j in range(3):
                t = vt_pool.tile([128, 2 * QB], dt.float32, tag=f"vT{j}",
                                 name=f"vT{j}_{qb}")
                c0 = qb * HT * QB + j * 2 * QB
                eng.dma_start(t[:], vT_ext[:, c0: c0 + 2 * QB])
                tiles.append(t)
            return tiles

        state = {}
        # vT(0) descgen on the Scalar queue now, before attention exps occupy
        # the Scalar sequencer
        state[0] = {"vT": load_vt(0, nc.scalar)}
        for ot in range(HT):
            q_tile(0, qx0, ot)

        def attn_block(qb, filler):
            """Merged scores+PV for q-block qb; filler(slot) emits interleaved
            PE work (proj/MLP tiles) - called with slot index 0..7 per jk."""
            st = state.setdefault(qb, {})
            if "vT" not in st:
                st["vT"] = load_vt(qb, nc.sync)
            sum_a = st_pool.tile([128, QB], dt.float32r, tag="sum_a",
                                 name=f"sum_a{qb}")
            sum_b = st_pool.tile([128, QB], dt.float32r, tag="sum_b",
                                 name=f"sum_b{qb}")
            ps_w = [ps_pool.tile([128, QB], dt.float32, tag=f"ps_w{ht}",
                                 name=f"ps_w{ht}_{qb}")
                    for ht in range(HT)]
            exp_pairs = []
            for jk in range(KTILES // 2):
                pair = exp_pool.tile([128, 2 * QB], FP8, tag="expS",
                                     name=f"expS_{qb}_{jk}")
                exp_pairs.append(pair)
                for t2 in range(2):
                    kt = 2 * jk + t2
                    ps_s = ps_gen.tile([128, QB], dt.float32, tag="gen",
                                       name=f"ps_s_{qb}_{kt}")
                    for jo in range(3):
                        nc.tensor.matmul(
                            ps_s[:],
                            KT3[kt // 4][:, 2 * jo: 2 * jo + 2,
                                         (kt % 4) * 128:(kt % 4 + 1) * 128],
                            QT[qb][jo][:].rearrange("p (t q) -> p t q", t=2),
                            start=(jo == 0), stop=(jo == 2), perf_mode=PMODE,
                        )
                    half = pair[:, t2 * QB:(t2 + 1) * QB]
                    nc.scalar.activation(half, ps_s[:], AF.Exp,
                                         scale=1.0 / QSCALE)
                    # two DVE accumulators (GpSimd fp8 2-input ops measured
                    # far too slow); two chains halve the serial dependency
                    if kt == 0:
                        nc.vector.tensor_copy(sum_b[:], half)
                    elif kt == 1:
                        nc.vector.tensor_copy(sum_a[:], half)
                    elif kt % 2 == 0:
                        nc.vector.tensor_add(sum_b[:], sum_b[:], half)
                    else:
                        nc.vector.tensor_add(sum_a[:], sum_a[:], half)
                filler(qb, jk)
                if jk >= 1:
                    _pv_group(qb, jk - 1, exp_pairs[jk - 1], ps_w,
                              start=(jk == 1), stop=False)
            _pv_group(qb, 7, exp_pairs[7], ps_w, start=False, stop=True)
            st["ps_w"] = ps_w

            # rowsum (both accumulators) -> 1/x on [1,512] -> PE broadcast
            ps_sum = ps_gen.tile([1, QB], dt.float32, tag="gen",
                                 name=f"ps_sum{qb}")
            nc.tensor.matmul(ps_sum[:], ones_col[:], sum_a[:],
                             start=True, stop=False)
            nc.tensor.matmul(ps_sum[:], ones_col[:], sum_b[:],
                             start=False, stop=True)
            # 1/x as exp(-ln(x)) on ScalarE: DVE reciprocal costs 3.4us even
            # on [1,512] (per-lane serial), ScalarE runs [1,512] in ~0.7us/op
            # and can emit float32r for the broadcast matmul directly
            lnd = st_pool.tile([1, QB], dt.float32, tag="lnd",
                               name=f"lnd{qb}")
            nc.scalar.activation(lnd[:], ps_sum[:], AF.Ln)
            rsum_r = st_pool.tile([1, QB], dt.float32r, tag="rsum",
                                  name=f"rsum{qb}")
            nc.scalar.activation(rsum_r[:], lnd[:], AF.Exp, scale=-1.0)
            filler(qb, 8)   # PE work while ScalarE runs ln/exp
            ps_b = ps_gen.tile([128, QB], dt.float32, tag="gen",
                               name=f"ps_b{qb}")
            nc.tensor.matmul(ps_b[:], ones_row[:], rsum_r[:],
                             start=True, stop=True)
            bcast = st_pool.tile([128, QB], dt.float32, tag="bcast",
                                 name=f"bcast{qb}")
            nc.scalar.copy(bcast[:], ps_b[:])
            filler(qb, 9)   # PE work while the broadcast is copied out
            st["bcast"] = bcast

        def _pv_group(qb, jk, pair, ps_w, start, stop):
            rhs = pair[:].rearrange("p (t q) -> p t q", t=2)
            for ht in range(HT):
                nc.tensor.matmul(ps_w[ht][:], vpair(jk, ht), rhs,
                                 start=start, stop=stop, perf_mode=PMODE)

        def weighted(qb):
            """weighted^T = PV * bcast + value^T; bf16 store + fp8 copy."""
            st = state[qb]
            wT_bf, wT_f8 = [], []
            for j in range(3):
                wT_bf.append(wt_pool.tile([128, 2 * QB], dt.bfloat16,
                                          tag=f"wTb{j}", name=f"wTb{j}_{qb}"))
                wT_f8.append(wt_pool.tile([128, 2 * QB], FP8,
                                          tag=f"wT8{j}", name=f"wT8{j}_{qb}"))
            for ht in range(HT):
                j, h2 = divmod(ht, 2)
                tmp = tmp_pool.tile([128, QB], dt.float32, tag="wtmp",
                                    name=f"wtmp_{qb}_{ht}")
                nc.vector.tensor_mul(tmp[:], st["ps_w"][ht][:], st["bcast"][:])
                nc.vector.tensor_add(
                    wT_bf[j][:, h2 * QB:(h2 + 1) * QB], tmp[:],
                    st["vT"][j][:, h2 * QB:(h2 + 1) * QB],
                )
                if h2 == 1:
                    nc.vector.tensor_copy(wT_f8[j][:], wT_bf[j][:])
            st["wT_bf"] = wT_bf
            st["wT_f8"] = wT_f8

        def mlp_h1_tile(qb, ot, rot=False):
            st = state[qb]
            if "h1" not in st:
                st["h1"] = [h1_pool.tile([128, 2 * QB], FP8, tag=f"h1_{j}",
                                         name=f"h1_{j}_{qb}")
                            for j in range(3)]
            if rot:
                ps = proj_ps(f"ps_h1_{qb}_{ot}")
            else:
                ps = ps_gen.tile([128, QB], dt.float32, tag="gen",
                                 name=f"ps_h1_{qb}_{ot}")
            for j in range(3):
                nc.tensor.matmul(
                    ps[:], wpair("wo1T", ot, j),
                    st["wT_f8"][j][:].rearrange("p (t q) -> p t q", t=2),
                    start=(j == 0), stop=(j == 2), perf_mode=PMODE,
                )
            j, h2 = divmod(ot, 2)
            nc.scalar.activation(
                st["h1"][j][:, h2 * QB:(h2 + 1) * QB], ps[:], AF.Relu,
                bias=biases["bo1"][:, ot: ot + 1], scale=1.0 / WSCALE,
            )

        def mlp_out_tile(qb, ot, rot=False):
            st = state[qb]
            if rot:
                ps = proj_ps(f"ps_o_{qb}_{ot}")
            else:
                ps = ps_gen.tile([128, QB], dt.float32, tag="gen",
                                 name=f"ps_o_{qb}_{ot}")
            for j in range(3):
                nc.tensor.matmul(
                    ps[:], wpair("wo2T", ot, j),
                    st["h1"][j][:].rearrange("p (t q) -> p t q", t=2),
                    start=(j == 0), stop=False, perf_mode=PMODE,
                )
            # residual rides the PE: PSUM += 64*I @ weighted^T (bf16, exact),
            # so the eviction is a single ScalarE op (scale 1/64 + bias) and
            # no DVE add sits on the output critical path
            j, h2 = divmod(ot, 2)
            nc.tensor.matmul(
                ps[:], id64_sb[:],
                st["wT_bf"][j][:, h2 * QB:(h2 + 1) * QB],
                start=False, stop=True,
            )
            o_sb = o_pool.tile([128, QB], dt.float32, tag="outT_blk",
                               name=f"outT_{qb}_{ot}")
            if ot % 2 == 0:
                nc.scalar.activation(o_sb[:], ps[:], AF.Identity,
                                     bias=biases["bo2"][:, ot: ot + 1],
                                     scale=1.0 / WSCALE)
            else:
                nc.vector.tensor_scalar(o_sb[:], ps[:], 1.0 / WSCALE,
                                        biases["bo2"][:, ot: ot + 1],
                                        mybir.AluOpType.mult,
                                        mybir.AluOpType.add)
            eng = nc.sync if ot % 2 == 0 else nc.scalar
            eng.dma_start(
                outT_ext[:, (qb * HT + ot) * QB:(qb * HT + ot + 1) * QB],
                o_sb[:],
            )

        # ---- attention block 0: interleave Q-proj nb=1 tiles; slots 8/9
        # bracket the softmax normalization chain with PE work ----
        def fill0(qb, jk):
            if jk in (0, 1, 2):
                k_tile(3, kx3_saved, 2 * jk)
                k_tile(3, kx3_saved, 2 * jk + 1)
            elif jk in (5, 6, 7):
                q_tile(1, qx1, jk - 5, gen_only=True)
            elif jk == 8:
                q_tile(1, qx1, 3, gen_only=True)
                q_tile(1, qx1, 4, gen_only=True)
            elif jk == 9:
                q_tile(1, qx1, 5, gen_only=True)

        attn_block(0, fill0)
        weighted(0)

        # ---- attention block 1: interleave MLP(0) h1 tiles ----
        def fill1(qb, jk):
            if jk in (5, 6, 7):
                mlp_h1_tile(0, jk - 5)
            elif jk == 8:
                mlp_h1_tile(0, 3)
                mlp_h1_tile(0, 4)
            elif jk == 9:
                mlp_h1_tile(0, 5)

        attn_block(1, fill1)
        # weighted(1) first: its DVE ops are the critical path into MLP(1);
        # mlp_out(0) and mlp_h1(1) alternate underneath it so the PE never
        # waits on a single eviction stream
        weighted(1)
        # the PE is in-order: emit all dependency-free mlp_out(0) tiles
        # before the first mlp_h1(1) tile (which waits for weighted(1)'s
        # first pair + fp8 cast). The MLP(1) tail rotates through the freed
        # PV PSUM banks so evictions never stall the 2-bank gen rotation.
        for ot in range(HT):
            mlp_out_tile(0, ot)
        for ot in range(HT):
            mlp_h1_tile(1, ot, rot=True)
        for ot in range(HT):
            mlp_out_tile(1, ot, rot=True)


# ---- host-side shard packing ----

def _tile_rows(a):
    """[T*128, N] -> [128, T*N]: partition-tiled T-layout, contiguous DMA."""
    t = a.shape[0] // 128
    return a.reshape(t, 128, a.shape[1]).transpose(1, 0, 2).reshape(128, -1)


def _tile_weight(w):
    """W^T [768h, 768o] -> [128, (ot, ht, 128)]: o-major packed lhsT tiles."""
    x = w.reshape(HT, 128, HT, 128)          # [ht, p, ot, o128]
    return x.transpose(1, 2, 0, 3).reshape(128, -1)


def _tile_rows_blocked(a, qb):
    """[768, NB*qb] -> [128, NB*(6*qb)]: per-block ht-major packing."""
    nb = a.shape[1] // qb
    x = a.reshape(HT, 128, nb, qb).transpose(1, 2, 0, 3)
    return x.reshape(128, -1)


def shard_inputs(query, key, value, Wq, bq, Wk, bk, Wo1, bo1, Wo2, bo2):
    """Full inputs -> per-core in_maps (host packing, fp8 cast, scale folds)."""
    scale = np.float32(1.0 / np.sqrt(np.float32(H)))

    def c8(x):
        return np.ascontiguousarray(
            np.clip(np.asarray(x, np.float32), -240, 240).astype(NP_FP8))

    def cb(x):
        return np.ascontiguousarray(np.asarray(x, np.float32).astype(NP_BF))

    def cf(x):
        return np.ascontiguousarray(x.astype(np.float32))

    shared = {
        "ident64": np.ascontiguousarray((np.eye(128, dtype=np.float32)
                                         * WSCALE).astype(NP_BF)),
        "wqT": c8(_tile_weight(Wq.T * (scale * QSCALE * WSCALE))),
        "wkT": c8(_tile_weight(Wk.T * WSCALE)),
        "wo1T": c8(_tile_weight(Wo1.T * WSCALE)),
        "wo2T": c8(_tile_weight(Wo2.T * WSCALE)),
        "biases": cf(np.concatenate([
            (bq * scale * QSCALE).reshape(HT, 128).T,
            bo1.reshape(HT, 128).T,
            bo2.reshape(HT, 128).T], axis=1)),
    }
    in_maps = []
    for core in range(N_CORES):
        b, half = divmod(core, 2)
        r0 = half * QCHUNK
        in_maps.append({
            "qT": c8(_tile_rows_blocked(query[b].T[:, r0: r0 + QCHUNK], QB)),
            "kT": c8(_tile_rows_blocked(key[b].T, QB)),
            "v": c8(_tile_rows(value[b])),
            "vT": cf(_tile_rows_blocked(value[b].T[:, r0: r0 + QCHUNK], QB)),
            **shared,
        })
    return in_maps


def gather_outputs(results):
    """Per-core outT [128, NQB*HT*QB] -> full [B, S, H]."""
    out = np.empty((B, S, H), dtype=np.float32)
    for core in range(N_CORES):
        b, half = divmod(core, 2)
        r0 = half * QCHUNK
        buf = results[core]["outT"].reshape(128, NQB, HT, QB)
        # out[q0+qb*QB+n, ot*128+p] = buf[p, qb, ot, n]
        out[b, r0: r0 + QCHUNK] = (
            buf.transpose(1, 3, 2, 0).reshape(QCHUNK, H)
        )
    return out


def run(inputs, trace=False):
    nc = build_kernel()
    in_maps = shard_inputs(**{k: np.asarray(v) for k, v in inputs.items()})
    res = run_bass_kernel_spmd(nc, in_maps, list(range(N_CORES)), trace=trace)
    return gather_outputs(res.results), res


def _split_multi_waits(nc):
    """Workaround for this container's walrus rejecting instructions that
    carry more than one semaphore wait ("Too many sync wait commands"):
    hoist N-1 waits onto fresh single-wait same-engine InstNoOp instructions
    inserted immediately before the instruction. Engine streams execute the
    block's per-engine subsequence in order, so blocking on the nops first is
    semantically identical to one multi-wait instruction."""
    for f in nc.m.functions:
        for bb in f.blocks:
            insts = list(bb.instructions)
            out = []
            changed = False
            for inst in insts:
                si = inst.sync_info
                waits = list(si.on_wait) if si is not None and si.on_wait else []
                if len(waits) > 1:
                    changed = True
                    for w in waits[:-1]:
                        nop = mybir.InstNoOp(
                            name=nc.get_next_instruction_name(), ins=[], outs=[]
                        )
                        nop.engine = inst.engine
                        nop.sync_info = mybir.SyncInfo(on_wait=[w], on_update=[])
                        out.append(nop)
                    si.on_wait = waits[-1:]
                    inst.sync_info = si
                out.append(inst)
            if changed:
                bb.instructions = out


def kernel(**inputs):
    """Entry point: full (unsharded) numpy inputs -> full [B, S, H] output."""
    out, _ = run(inputs, trace=False)
    return out


# revision 30
# speedup vs baseline: 1.0019x; 1.0019x over previous
"""Distributed single-head attention + MLP block for 8 TRN2 NeuronCores.

Reference computation (per batch b):
  Q = query @ Wq^T + bq ; K = key @ Wk^T + bk
  scores = Q @ K^T / sqrt(H) ; attn = softmax(scores)
  weighted = attn @ value + value
  h1 = relu(weighted @ Wo1^T + bo1)
  out = h1 @ Wo2^T + bo2 + weighted

Sharding: B=4 batches x 2 query-row halves = 8 shards. Each core gets its
1024 query rows plus the full 2048 keys/values of its batch; attention is
dense (non-causal) so no inter-core communication is needed.

Layout: everything on-device is feature-on-partitions ("T-layout",
X^T[f, tok]) so all matmul contractions line up with zero on-device
transposes; the host pre-packs every shard into the exact [128, free]
SBUF tiling the kernel consumes. All matmuls run fp8e4 DoubleRow (2x).

Numerics tricks (all folded on host / into activation scale operands):
  - softmax shift-invariance: K bias bk drops entirely (k-independent
    terms cancel; bq is kept on the Q side).
  - Q is stored as 32x true-Q and K as true-K so both live in fp8's
    normal range (raw Q*scale has std ~0.02 - subnormal); the scores
    PSUM is then 32x the true logits and the exp activation applies
    scale=1/32.
  - fp8 weights are stored 64x and unscaled via activation scale=1/64
    (uniform(-0.036, 0.036) weights would otherwise be ~half subnormal).
  - softmax needs no max-subtraction: logits have std ~1/3.

Schedule (PE never idles between phases):
  K-proj (24 tiles over an 8-bank PSUM rotation, DVE evictions)
  Q-proj nb=0, then attention block 0 with Q-proj nb=1 tiles interleaved
  attention block 1 with MLP(0) tiles interleaved, then MLP(1).
Attention merges scores and PV per k-tile pair: 3 score MMs -> exp on
ScalarE -> 6 PV MMs one pair behind, so PSUM-evict latency is always
covered by other matmuls. The softmax denominator accumulates on DVE
(even k-tiles) and GpSimd (odd k-tiles); the rowsum matmul sums both,
reciprocal runs on [1,512] only, and the PE broadcasts it back.
Bulk prefetches (v, vT, MLP weights) issue on the Scalar HWDGE queue
during the K-proj phase (ScalarE is idle there); everything else on Sync.
"""

import contextlib

import numpy as np
import ml_dtypes

import concourse.bass as bass
import concourse.mybir as mybir
import concourse.tile as tile
from concourse.bass_utils import run_bass_kernel_spmd

dt = mybir.dt
AF = mybir.ActivationFunctionType

H = 768          # model dim
B = 4            # batch
S = 2048         # sequence length
N_CORES = 8
QCHUNK = S * B // N_CORES        # 1024 query rows per core
HT = H // 128                    # 6 feature partition-tiles
KTILES = S // 128                # 16 key partition-tiles
QB = 512                         # q-block width (= PSUM bank, fp32)
NQB = QCHUNK // QB               # 2 q-blocks per core

FP8 = dt.float8e4
NP_FP8 = dt.np(FP8)
NP_BF = ml_dtypes.bfloat16
PMODE = mybir.MatmulPerfMode.DoubleRow

WSCALE = 64.0      # fp8 weight pre-scale (host) -> activation scale 1/64
QSCALE = 32.0      # stored Q = 32x true Q -> exp scale 1/32


def build_kernel():
    nc = bass.Bass()

    qT_ext = nc.declare_dram_parameter("qT", [128, NQB * HT * QB], FP8, isOutput=False)
    kT_ext = nc.declare_dram_parameter("kT", [128, (S // QB) * HT * QB], FP8, isOutput=False)
    v_ext = nc.declare_dram_parameter("v", [128, KTILES * H], FP8, isOutput=False)
    vT_ext = nc.declare_dram_parameter("vT", [128, NQB * HT * QB], dt.float32, isOutput=False)
    w_ext = {
        name: nc.declare_dram_parameter(name, [128, HT * H], FP8, isOutput=False)
        for name in ("wqT", "wkT", "wo1T", "wo2T")
    }
    b_ext = nc.declare_dram_parameter("biases", [128, 3 * HT], dt.float32,
                                      isOutput=False)
    id64_ext = nc.declare_dram_parameter("ident64", [128, 128], dt.bfloat16,
                                         isOutput=False)
    outT_ext = nc.declare_dram_parameter(
        "outT", [128, NQB * HT * QB], dt.float32, isOutput=True
    )

    with tile.TileContext(nc) as tc, nc.allow_low_precision(
        reason="fp8 matmul path is intentional; rel-err budget is 2e-2"
    ):
        _body(nc, tc, qT_ext, kT_ext, v_ext, vT_ext, w_ext, b_ext, id64_ext,
              outT_ext)

    _split_multi_waits(nc)
    return nc


def _body(nc, tc, qT_ext, kT_ext, v_ext, vT_ext, w_ext, b_ext, id64_ext,
          outT_ext):
    with contextlib.ExitStack() as ctx:
        const_pool = ctx.enter_context(tc.tile_pool(name="const", bufs=1))
        w_pool = ctx.enter_context(tc.tile_pool(name="w", bufs=1))
        act_pool = ctx.enter_context(tc.tile_pool(name="act", bufs=1))
        in_pool = ctx.enter_context(tc.tile_pool(name="inp", bufs=2))
        vt_pool = ctx.enter_context(tc.tile_pool(name="vt", bufs=2))
        wt_pool = ctx.enter_context(tc.tile_pool(name="wt", bufs=2))
        h1_pool = ctx.enter_context(tc.tile_pool(name="h1", bufs=2))
        st_pool = ctx.enter_context(tc.tile_pool(name="st", bufs=2))
        tmp_pool = ctx.enter_context(tc.tile_pool(name="tmp", bufs=4))
        o_pool = ctx.enter_context(tc.tile_pool(name="o", bufs=6))
        exp_pool = ctx.enter_context(tc.tile_pool(name="exps", bufs=6))
        # PSUM: 6 PV accumulators + 2 general banks = 8 banks exactly.
        ps_pool = ctx.enter_context(tc.tile_pool(name="ps", bufs=1, space="PSUM"))
        ps_gen = ctx.enter_context(tc.tile_pool(name="ps_gen", bufs=2, space="PSUM"))

        # ---- weight / bias loading helpers ----
        w_sb = {}

        def load_weight_chunk(name, j, eng=None):
            ts = w_sb.get(name)
            if ts is None:
                ts = [w_pool.tile([128, HT * 128], FP8, tag=f"{name}{g}",
                                  name=f"w_{name}{g}") for g in range(HT)]
                w_sb[name] = ts
            step = HT * 128
            (eng or nc.sync).dma_start(ts[j][:],
                                       w_ext[name][:, j * step:(j + 1) * step])

        def wpair(name, ot, j):
            """lhsT [128, 2, 128]: out-tile ot, contraction h-tile pair (2j, 2j+1)."""
            c0 = 2 * j * 128
            return (w_sb[name][ot][:, c0: c0 + 256]
                    .rearrange("p (t m) -> p t m", t=2))

        # ---- stage-1 DMAs in first-use order (Sync queue) ----
        def load_x_chunks(ext, nb, tagbase):
            """One projection input block as 3 ht-pair tiles [128, 2*QB]."""
            tiles = []
            for j in range(3):
                t = in_pool.tile([128, 2 * QB], FP8, tag=f"{tagbase}{j}",
                                 name=f"x_{tagbase}{j}_{nb}")
                c0 = nb * HT * QB + j * 2 * QB
                nc.sync.dma_start(t[:], ext[:, c0: c0 + 2 * QB])
                tiles.append(t)
            return tiles

        load_weight_chunk("wkT", 0)
        kx0 = []
        for j in range(3):
            t = in_pool.tile([128, 2 * QB], FP8, tag=f"kx{j}", name=f"x_kx{j}_0")
            nc.sync.dma_start(t[:], kT_ext[:, j * 2 * QB:(j + 1) * 2 * QB])
            kx0.append(t)
        for j in range(1, HT):
            load_weight_chunk("wkT", j, eng=nc.scalar)

        bias_sb = const_pool.tile([128, 3 * HT], dt.float32, tag="biases")
        nc.sync.dma_start(bias_sb[:], b_ext[:])
        id64_sb = const_pool.tile([128, 128], dt.bfloat16, tag="ident64")
        nc.scalar.dma_start(id64_sb[:], id64_ext[:])
        biases = {name: bias_sb[:, i * HT:(i + 1) * HT]
                  for i, name in enumerate(("bq", "bo1", "bo2"))}

        # ---- activation tiles ----
        KT = [act_pool.tile([128, HT * QB], FP8, tag=f"KT{nb}",
                            name=f"KT{nb}") for nb in range(4)]
        KT3 = [t[:].rearrange("p (t k) -> p t k", t=HT) for t in KT]
        QT = [[act_pool.tile([128, 2 * QB], FP8, tag=f"QT{qb}_{j}",
                             name=f"QT{qb}_{j}") for j in range(3)]
              for qb in range(NQB)]

        ones_f32 = const_pool.tile([128, 128], dt.float32, tag="ones_f32")
        nc.vector.memset(ones_f32[:], 1.0)
        ones_row = const_pool.tile([1, 128], dt.float32r, tag="ones_row")
        nc.vector.tensor_copy(ones_row[:], ones_f32[0:1, :])
        ones_col = const_pool.tile([128, 1], dt.float32r, tag="ones_col")
        nc.vector.tensor_copy(ones_col[:], ones_f32[:, 0:1])

        # 8-bank PSUM rotation for the projection phase (PV banks are free).
        _rot = {"i": 0}

        def proj_ps(nm):
            i = _rot["i"]
            _rot["i"] += 1
            if i % 8 < 2:
                return ps_gen.tile([128, QB], dt.float32, tag="gen", name=nm)
            return ps_pool.tile([128, QB], dt.float32, tag=f"ps_w{i % 8 - 2}",
                                name=nm)

        def proj_tile(wname, xt, ot, ps):
            """3 DoubleRow matmuls: one [128out, QB] projection tile."""
            for j in range(3):
                nc.tensor.matmul(
                    ps[:], wpair(wname, ot, j),
                    xt[j][:].rearrange("p (t q) -> p t q", t=2),
                    start=(j == 0), stop=(j == 2), perf_mode=PMODE,
                )

        # ---- K projection: 24 tiles, DVE evictions (no bias - bk drops) ----
        k_tile_fns = []

        def k_tile(nb, xt, ot):
            ps = ps_gen.tile([128, QB], dt.float32, tag="gen",
                             name=f"ps_k3_{ot}")
            proj_tile("wkT", xt, ot, ps)
            nc.vector.tensor_scalar_mul(KT[nb][:, ot * QB:(ot + 1) * QB],
                                        ps[:], 1.0 / WSCALE)

        cur = kx0
        for nb in range(4):
            nxt = load_x_chunks(kT_ext, nb + 1, "kx") if nb < 3 else None
            if nb == 3:
                # nb3 tiles are emitted inside attention block 0 (fill slots)
                kx3_saved = cur
                break
            if nb == 1:
                v_blks = []
                for c in range(4):
                    t = act_pool.tile([128, 4 * H], FP8, tag=f"v_in{c}",
                                      name=f"v_in{c}")
                    nc.scalar.dma_start(t[:], v_ext[:, c * 4 * H:(c + 1) * 4 * H])
                    v_blks.append(t)
            elif nb == 2:
                pass
            for ot in range(HT):
                ps = proj_ps(f"ps_k_{nb}_{ot}")
                proj_tile("wkT", cur, ot, ps)
                dst = KT[nb][:, ot * QB:(ot + 1) * QB]
                # all evictions on DVE: anything on the Scalar stream lands
                # behind DIRECT2D descriptor-generation bursts and stalls
                # the PSUM rotation
                nc.vector.tensor_scalar_mul(dst, ps[:], 1.0 / WSCALE)
            cur = nxt

        def vpair(jk, ht):
            """lhsT [128, 2, 128]: k-tile pair (2jk, 2jk+1), h-tile ht."""
            t = v_blks[jk // 2]
            j2 = (jk % 2) * 2
            return (t[:].rearrange("p (t h) -> p t h", t=4)
                    [:, j2: j2 + 2, ht * 128:(ht + 1) * 128])

        # ---- Q projection nb=0 (ScalarE evictions apply bias + 1/64) ----
        for j in range(HT):
            load_weight_chunk("wqT", j)
        qx0 = load_x_chunks(qT_ext, 0, "qx")
        qx1 = load_x_chunks(qT_ext, 1, "qx")
        for j in range(HT):
            load_weight_chunk("wo1T", j)
        for j in range(HT):
            load_weight_chunk("wo2T", j)

        def q_tile(qb, xt, ot, gen_only=False):
            # interleaved tiles (inside attention) must not touch the live
            # PV accumulator banks - gen rotation only
            if gen_only:
                ps = ps_gen.tile([128, QB], dt.float32, tag="gen",
                                 name=f"ps_q_{qb}_{ot}")
            else:
                ps = proj_ps(f"ps_q_{qb}_{ot}")
            proj_tile("wqT", xt, ot, ps)
            nc.scalar.activation(
                QT[qb][ot // 2][:, (ot % 2) * QB:(ot % 2 + 1) * QB], ps[:],
                AF.Identity,
                bias=biases["bq"][:, ot: ot + 1], scale=1.0 / WSCALE,
            )

        # ---- vT (bf16 residual) as ht-pair tiles ----
        def load_vt(qb, eng):
            tiles = []
            for j in range(3):
                t = vt_pool.tile([128, 2 * QB], dt.float32, tag=f"vT{j}",
                                 name=f"vT{j}_{qb}")
                c0 = qb * HT * QB + j * 2 * QB
                eng.dma_start(t[:], vT_ext[:, c0: c0 + 2 * QB])
                tiles.append(t)
            return tiles

        state = {}
        # vT(0) descgen on the Scalar queue now, before attention exps occupy
        # the Scalar sequencer
        state[0] = {"vT": load_vt(0, nc.scalar)}
        for ot in range(HT):
            q_tile(0, qx0, ot)

        def attn_block(qb, filler):
            """Merged scores+PV for q-block qb; filler(slot) emits interleaved
            PE work (proj/MLP tiles) - called with slot index 0..7 per jk."""
            st = state.setdefault(qb, {})
            if "vT" not in st:
                st["vT"] = load_vt(qb, nc.sync)
            sum_a = st_pool.tile([128, QB], dt.float32r, tag="sum_a",
                                 name=f"sum_a{qb}")
            sum_b = st_pool.tile([128, QB], dt.float32r, tag="sum_b",
                                 name=f"sum_b{qb}")
            ps_w = [ps_pool.tile([128, QB], dt.float32, tag=f"ps_w{ht}",
                                 name=f"ps_w{ht}_{qb}")
                    for ht in range(HT)]
            exp_pairs = []
            for jk in range(KTILES // 2):
                pair = exp_pool.tile([128, 2 * QB], FP8, tag="expS",
                                     name=f"expS_{qb}_{jk}")
                exp_pairs.append(pair)
                for t2 in range(2):
                    kt = 2 * jk + t2
                    ps_s = ps_gen.tile([128, QB], dt.float32, tag="gen",
                                       name=f"ps_s_{qb}_{kt}")
                    for jo in range(3):
                        nc.tensor.matmul(
                            ps_s[:],
                            KT3[kt // 4][:, 2 * jo: 2 * jo + 2,
                                         (kt % 4) * 128:(kt % 4 + 1) * 128],
                            QT[qb][jo][:].rearrange("p (t q) -> p t q", t=2),
                            start=(jo == 0), stop=(jo == 2), perf_mode=PMODE,
                        )
                    half = pair[:, t2 * QB:(t2 + 1) * QB]
                    nc.scalar.activation(half, ps_s[:], AF.Exp,
                                         scale=1.0 / QSCALE)
                    # two DVE accumulators (GpSimd fp8 2-input ops measured
                    # far too slow); two chains halve the serial dependency
                    if kt == 0:
                        nc.vector.tensor_copy(sum_b[:], half)
                    elif kt == 1:
                        nc.vector.tensor_copy(sum_a[:], half)
                    elif kt % 2 == 0:
                        nc.vector.tensor_add(sum_b[:], sum_b[:], half)
                    else:
                        nc.vector.tensor_add(sum_a[:], sum_a[:], half)
                filler(qb, jk)
                if jk >= 1:
                    _pv_group(qb, jk - 1, exp_pairs[jk - 1], ps_w,
                              start=(jk == 1), stop=False)
            _pv_group(qb, 7, exp_pairs[7], ps_w, start=False, stop=True)
            st["ps_w"] = ps_w

            # rowsum (both accumulators) -> 1/x on [1,512] -> PE broadcast
            ps_sum = ps_gen.tile([1, QB], dt.float32, tag="gen",
                                 name=f"ps_sum{qb}")
            nc.tensor.matmul(ps_sum[:], ones_col[:], sum_a[:],
                             start=True, stop=False)
            nc.tensor.matmul(ps_sum[:], ones_col[:], sum_b[:],
                             start=False, stop=True)
            # 1/x as exp(-ln(x)) on ScalarE: DVE reciprocal costs 3.4us even
            # on [1,512] (per-lane serial), ScalarE runs [1,512] in ~0.7us/op
            # and can emit float32r for the broadcast matmul directly
            lnd = st_pool.tile([1, QB], dt.float32, tag="lnd",
                               name=f"lnd{qb}")
            nc.scalar.activation(lnd[:], ps_sum[:], AF.Ln)
            rsum_r = st_pool.tile([1, QB], dt.float32r, tag="rsum",
                                  name=f"rsum{qb}")
            nc.scalar.activation(rsum_r[:], lnd[:], AF.Exp, scale=-1.0)
            filler(qb, 8)   # PE work while ScalarE runs ln/exp
            ps_b = ps_gen.tile([128, QB], dt.float32, tag="gen",
                               name=f"ps_b{qb}")
            nc.tensor.matmul(ps_b[:], ones_row[:], rsum_r[:],
                             start=True, stop=True)
            bcast = st_pool.tile([128, QB], dt.float32, tag="bcast",
                                 name=f"bcast{qb}")
            nc.scalar.copy(bcast[:], ps_b[:])
            filler(qb, 9)   # PE work while the broadcast is copied out
            st["bcast"] = bcast

        def _pv_group(qb, jk, pair, ps_w, start, stop):
            rhs = pair[:].rearrange("p (t q) -> p t q", t=2)
            for ht in range(HT):
                nc.tensor.matmul(ps_w[ht][:], vpair(jk, ht), rhs,
                                 start=start, stop=stop, perf_mode=PMODE)

        def weighted(qb):
            """weighted^T = PV * bcast + value^T; bf16 store + fp8 copy."""
            st = state[qb]
            wT_bf, wT_f8 = [], []
            for j in range(3):
                wT_bf.append(wt_pool.tile([128, 2 * QB], dt.bfloat16,
                                          tag=f"wTb{j}", name=f"wTb{j}_{qb}"))
                wT_f8.append(wt_pool.tile([128, 2 * QB], FP8,
                                          tag=f"wT8{j}", name=f"wT8{j}_{qb}"))
            for ht in range(HT):
                j, h2 = divmod(ht, 2)
                tmp = tmp_pool.tile([128, QB], dt.float32, tag="wtmp",
                                    name=f"wtmp_{qb}_{ht}")
                nc.vector.tensor_mul(tmp[:], st["ps_w"][ht][:], st["bcast"][:])
                nc.vector.tensor_add(
                    wT_bf[j][:, h2 * QB:(h2 + 1) * QB], tmp[:],
                    st["vT"][j][:, h2 * QB:(h2 + 1) * QB],
                )
                if h2 == 1:
                    nc.vector.tensor_copy(wT_f8[j][:], wT_bf[j][:])
            st["wT_bf"] = wT_bf
            st["wT_f8"] = wT_f8

        def mlp_h1_tile(qb, ot, rot=False):
            st = state[qb]
            if "h1" not in st:
                st["h1"] = [h1_pool.tile([128, 2 * QB], FP8, tag=f"h1_{j}",
                                         name=f"h1_{j}_{qb}")
                            for j in range(3)]
            if rot:
                ps = proj_ps(f"ps_h1_{qb}_{ot}")
            else:
                ps = ps_gen.tile([128, QB], dt.float32, tag="gen",
                                 name=f"ps_h1_{qb}_{ot}")
            for j in range(3):
                nc.tensor.matmul(
                    ps[:], wpair("wo1T", ot, j),
                    st["wT_f8"][j][:].rearrange("p (t q) -> p t q", t=2),
                    start=(j == 0), stop=(j == 2), perf_mode=PMODE,
                )
            j, h2 = divmod(ot, 2)
            nc.scalar.activation(
                st["h1"][j][:, h2 * QB:(h2 + 1) * QB], ps[:], AF.Relu,
                bias=biases["bo1"][:, ot: ot + 1], scale=1.0 / WSCALE,
            )

        def mlp_out_tile(qb, ot, rot=False):
            st = state[qb]
            if rot:
                ps = proj_ps(f"ps_o_{qb}_{ot}")
            else:
                ps = ps_gen.tile([128, QB], dt.float32, tag="gen",
                                 name=f"ps_o_{qb}_{ot}")
            for j in range(3):
                nc.tensor.matmul(
                    ps[:], wpair("wo2T", ot, j),
                    st["h1"][j][:].rearrange("p (t q) -> p t q", t=2),
                    start=(j == 0), stop=False, perf_mode=PMODE,
                )
            # residual rides the PE: PSUM += 64*I @ weighted^T (bf16, exact),
            # so the eviction is a single ScalarE op (scale 1/64 + bias) and
            # no DVE add sits on the output critical path
            j, h2 = divmod(ot, 2)
            nc.tensor.matmul(
                ps[:], id64_sb[:],
                st["wT_bf"][j][:, h2 * QB:(h2 + 1) * QB],
                start=False, stop=True,
            )
            o_sb = o_pool.tile([128, QB], dt.float32, tag="outT_blk",
                               name=f"outT_{qb}_{ot}")
            if ot % 2 == 0:
                nc.scalar.activation(o_sb[:], ps[:], AF.Identity,
                                     bias=biases["bo2"][:, ot: ot + 1],
                                     scale=1.0 / WSCALE)
            else:
                nc.vector.tensor_scalar(o_sb[:], ps[:], 1.0 / WSCALE,
                                        biases["bo2"][:, ot: ot + 1],
                                        mybir.AluOpType.mult,
                                        mybir.AluOpType.add)
            eng = nc.sync if ot % 2 == 0 else nc.scalar
            eng.dma_start(
                outT_ext[:, (qb * HT + ot) * QB:(qb * HT + ot + 1) * QB],
                o_sb[:],
            )

        # ---- attention block 0: interleave Q-proj nb=1 tiles; slots 8/9
        # bracket the softmax normalization chain with PE work ----
        def fill0(qb, jk):
            if jk in (0, 1, 2):
                k_tile(3, kx3_saved, 2 * jk)
                k_tile(3, kx3_saved, 2 * jk + 1)
            elif jk in (5, 6, 7):
                q_tile(1, qx1, jk - 5, gen_only=True)
            elif jk == 8:
                q_tile(1, qx1, 3, gen_only=True)
                q_tile(1, qx1, 4, gen_only=True)
            elif jk == 9:
                q_tile(1, qx1, 5, gen_only=True)

        attn_block(0, fill0)
        weighted(0)

        # ---- attention block 1: interleave MLP(0) h1 tiles ----
        def fill1(qb, jk):
            if jk in (5, 6, 7):
                mlp_h1_tile(0, jk - 5)
            elif jk == 8:
                mlp_h1_tile(0, 3)
                mlp_h1_tile(0, 4)
            elif jk == 9:
                mlp_h1_tile(0, 5)

        attn_block(1, fill1)
        # weighted(1) first: its DVE ops are the critical path into MLP(1);
        # mlp_out(0) and mlp_h1(1) alternate underneath it so the PE never
        # waits on a single eviction stream
        weighted(1)
        # the PE is in-order: emit all dependency-free mlp_out(0) tiles
        # before the first mlp_h1(1) tile (which waits for weighted(1)'s
        # first pair + fp8 cast). The MLP(1) tail rotates through the freed
        # PV PSUM banks so evictions never stall the 2-bank gen rotation.
        for ot in range(HT):
            mlp_out_tile(0, ot)
        for ot in range(HT):
            mlp_h1_tile(1, ot, rot=True)
        for ot in range(HT):
            mlp_out_tile(1, ot, rot=True)


# ---- host-side shard packing ----

def _tile_rows(a):
    """[T*128, N] -> [128, T*N]: partition-tiled T-layout, contiguous DMA."""
    t = a.shape[0] // 128
    return a.reshape(t, 128, a.shape[1]).transpose(1, 0, 2).reshape(128, -1)


def _tile_weight(w):
    """W^T [768h, 768o] -> [128, (ot, ht, 128)]: o-major packed lhsT tiles."""
    x = w.reshape(HT, 128, HT, 128)          # [ht, p, ot, o128]
    return x.transpose(1, 2, 0, 3).reshape(128, -1)


def _tile_rows_blocked(a, qb):
    """[768, NB*qb] -> [128, NB*(6*qb)]: per-block ht-major packing."""
    nb = a.shape[1] // qb
    x = a.reshape(HT, 128, nb, qb).transpose(1, 2, 0, 3)
    return x.reshape(128, -1)


def shard_inputs(query, key, value, Wq, bq, Wk, bk, Wo1, bo1, Wo2, bo2):
    """Full inputs -> per-core in_maps (host packing, fp8 cast, scale folds)."""
    scale = np.float32(1.0 / np.sqrt(np.float32(H)))

    def c8(x):
        return np.ascontiguousarray(
            np.clip(np.asarray(x, np.float32), -240, 240).astype(NP_FP8))

    def cb(x):
        return np.ascontiguousarray(np.asarray(x, np.float32).astype(NP_BF))

    def cf(x):
        return np.ascontiguousarray(x.astype(np.float32))

    shared = {
        "ident64": np.ascontiguousarray((np.eye(128, dtype=np.float32)
                                         * WSCALE).astype(NP_BF)),
        "wqT": c8(_tile_weight(Wq.T * (scale * QSCALE * WSCALE))),
        "wkT": c8(_tile_weight(Wk.T * WSCALE)),
        "wo1T": c8(_tile_weight(Wo1.T * WSCALE)),
        "wo2T": c8(_tile_weight(Wo2.T * WSCALE)),
        "biases": cf(np.concatenate([
            (bq * scale * QSCALE).reshape(HT, 128).T,
            bo1.reshape(HT, 128).T,
            bo2.reshape(HT, 128).T], axis=1)),
    }
    in_maps = []
    for core in range(N_CORES):
        b, half = divmod(core, 2)
        r0 = half * QCHUNK
        in_maps.append({
            "qT": c8(_tile_rows_blocked(query[b].T[:, r0: r0 + QCHUNK], QB)),
            "kT": c8(_tile_rows_blocked(key[b].T, QB)),
            "v": c8(_tile_rows(value[b])),
            "vT": cf(_tile_rows_blocked(value[b].T[:, r0: r0 + QCHUNK], QB)),
            **shared,
        })
    return in_maps


def gather_outputs(results):
    """Per-core outT [128, NQB*HT*QB] -> full [B, S, H]."""
    out = np.empty((B, S, H), dtype=np.float32)
    for core in range(N_CORES):
        b, half = divmod(core, 2)
        r0 = half * QCHUNK
        buf = results[core]["outT"].reshape(128, NQB, HT, QB)
        # out[q0+qb*QB+n, ot*128+p] = buf[p, qb, ot, n]
        out[b, r0: r0 + QCHUNK] = (
            buf.transpose(1, 3, 2, 0).reshape(QCHUNK, H)
        )
    return out


def run(inputs, trace=False):
    nc = build_kernel()
    in_maps = shard_inputs(**{k: np.asarray(v) for k, v in inputs.items()})
    res = run_bass_kernel_spmd(nc, in_maps, list(range(N_CORES)), trace=trace)
    return gather_outputs(res.results), res


def _split_multi_waits(nc):
    """Workaround for this container's walrus rejecting instructions that
    carry more than one semaphore wait ("Too many sync wait commands"):
    hoist N-1 waits onto fresh single-wait same-engine InstNoOp instructions
    inserted immediately before the instruction. Engine streams execute the
    block's per-engine subsequence in order, so blocking on the nops first is
    semantically identical to one multi-wait instruction."""
    for f in nc.m.functions:
        for bb in f.blocks:
            insts = list(bb.instructions)
            out = []
            changed = False
            for inst in insts:
                si = inst.sync_info
                waits = list(si.on_wait) if si is not None and si.on_wait else []
                if len(waits) > 1:
                    changed = True
                    for w in waits[:-1]:
                        nop = mybir.InstNoOp(
                            name=nc.get_next_instruction_name(), ins=[], outs=[]
                        )
                        nop.engine = inst.engine
                        nop.sync_info = mybir.SyncInfo(on_wait=[w], on_update=[])
                        out.append(nop)
                    si.on_wait = waits[-1:]
                    inst.sync_info = si
                out.append(inst)
            if changed:
                bb.instructions = out


def kernel(**inputs):
    """Entry point: full (unsharded) numpy inputs -> full [B, S, H] output."""
    out, _ = run(inputs, trace=False)
    return out


# revision 31
# speedup vs baseline: 1.1026x; 1.1005x over previous
"""Distributed single-head attention + MLP block for 8 TRN2 NeuronCores.

Reference computation (per batch b):
  Q = query @ Wq^T + bq ; K = key @ Wk^T + bk
  scores = Q @ K^T / sqrt(H) ; attn = softmax(scores)
  weighted = attn @ value + value
  h1 = relu(weighted @ Wo1^T + bo1)
  out = h1 @ Wo2^T + bo2 + weighted

Sharding: B=4 batches x 2 query-row halves = 8 shards. Each core gets its
1024 query rows plus the full 2048 keys/values of its batch; attention is
dense (non-causal) so no inter-core communication is needed.

Layout: everything on-device is feature-on-partitions ("T-layout",
X^T[f, tok]) so all matmul contractions line up with zero on-device
transposes; the host pre-packs every shard into the exact [128, free]
SBUF tiling the kernel consumes. All matmuls run fp8e4 DoubleRow (2x).

Numerics tricks (all folded on host / into activation scale operands):
  - softmax shift-invariance: K bias bk drops entirely (k-independent
    terms cancel; bq is kept on the Q side).
  - Q is stored as 32x true-Q and K as true-K so both live in fp8's
    normal range (raw Q*scale has std ~0.02 - subnormal); the scores
    PSUM is then 32x the true logits and the exp activation applies
    scale=1/32.
  - fp8 weights are stored 64x and unscaled via activation scale=1/64
    (uniform(-0.036, 0.036) weights would otherwise be ~half subnormal).
  - softmax needs no max-subtraction: logits have std ~1/3.

Schedule (PE never idles between phases):
  K-proj (24 tiles over an 8-bank PSUM rotation, DVE evictions)
  Q-proj nb=0, then attention block 0 with Q-proj nb=1 tiles interleaved
  attention block 1 with MLP(0) tiles interleaved, then MLP(1).
Attention merges scores and PV per k-tile pair: 3 score MMs -> exp on
ScalarE -> 6 PV MMs one pair behind, so PSUM-evict latency is always
covered by other matmuls. The softmax denominator accumulates on DVE
(even k-tiles) and GpSimd (odd k-tiles); the rowsum matmul sums both,
reciprocal runs on [1,512] only, and the PE broadcasts it back.
Bulk prefetches (v, vT, MLP weights) issue on the Scalar HWDGE queue
during the K-proj phase (ScalarE is idle there); everything else on Sync.
"""

import contextlib

import numpy as np
import ml_dtypes

import concourse.bass as bass
import concourse.mybir as mybir
import concourse.tile as tile
from concourse.bass_utils import run_bass_kernel_spmd

dt = mybir.dt
AF = mybir.ActivationFunctionType

H = 768          # model dim
B = 4            # batch
S = 2048         # sequence length
N_CORES = 8
QCHUNK = S * B // N_CORES        # 1024 query rows per core
HT = H // 128                    # 6 feature partition-tiles
KTILES = S // 128                # 16 key partition-tiles
QB = 512                         # q-block width (= PSUM bank, fp32)
NQB = QCHUNK // QB               # 2 q-blocks per core

FP8 = dt.float8e4
NP_FP8 = dt.np(FP8)
NP_BF = ml_dtypes.bfloat16
PMODE = mybir.MatmulPerfMode.DoubleRow

WSCALE = 64.0      # fp8 weight pre-scale (host) -> activation scale 1/64
QSCALE = 32.0      # stored Q = 32x true Q -> exp scale 1/32


def build_kernel():
    nc = bass.Bass()

    qT_ext = nc.declare_dram_parameter("qT", [128, NQB * HT * QB], FP8, isOutput=False)
    kT_ext = nc.declare_dram_parameter("kT", [128, (S // QB) * HT * QB], FP8, isOutput=False)
    v_ext = nc.declare_dram_parameter("v", [128, KTILES * H], FP8, isOutput=False)
    vT_ext = nc.declare_dram_parameter("vT", [128, NQB * HT * QB], dt.float32, isOutput=False)
    w_ext = {
        name: nc.declare_dram_parameter(name, [128, HT * H], FP8, isOutput=False)
        for name in ("wqT", "wkT", "wo1T", "wo2T")
    }
    b_ext = nc.declare_dram_parameter("biases", [128, 3 * HT], dt.float32,
                                      isOutput=False)
    id64_ext = nc.declare_dram_parameter("ident64", [128, 128], dt.bfloat16,
                                         isOutput=False)
    outT_ext = nc.declare_dram_parameter(
        "outT", [128, NQB * HT * QB], dt.float32, isOutput=True
    )

    with tile.TileContext(nc) as tc, nc.allow_low_precision(
        reason="fp8 matmul path is intentional; rel-err budget is 2e-2"
    ):
        _body(nc, tc, qT_ext, kT_ext, v_ext, vT_ext, w_ext, b_ext, id64_ext,
              outT_ext)

    _split_multi_waits(nc)
    return nc


def _body(nc, tc, qT_ext, kT_ext, v_ext, vT_ext, w_ext, b_ext, id64_ext,
          outT_ext):
    with contextlib.ExitStack() as ctx:
        const_pool = ctx.enter_context(tc.tile_pool(name="const", bufs=1))
        w_pool = ctx.enter_context(tc.tile_pool(name="w", bufs=1))
        act_pool = ctx.enter_context(tc.tile_pool(name="act", bufs=1))
        in_pool = ctx.enter_context(tc.tile_pool(name="inp", bufs=2))
        vt_pool = ctx.enter_context(tc.tile_pool(name="vt", bufs=2))
        wt_pool = ctx.enter_context(tc.tile_pool(name="wt", bufs=2))
        h1_pool = ctx.enter_context(tc.tile_pool(name="h1", bufs=2))
        st_pool = ctx.enter_context(tc.tile_pool(name="st", bufs=2))
        tmp_pool = ctx.enter_context(tc.tile_pool(name="tmp", bufs=4))
        o_pool = ctx.enter_context(tc.tile_pool(name="o", bufs=6))
        exp_pool = ctx.enter_context(tc.tile_pool(name="exps", bufs=6))
        # PSUM: 6 PV accumulators + 2 general banks = 8 banks exactly.
        ps_pool = ctx.enter_context(tc.tile_pool(name="ps", bufs=1, space="PSUM"))
        ps_gen = ctx.enter_context(tc.tile_pool(name="ps_gen", bufs=2, space="PSUM"))

        # ---- weight / bias loading helpers ----
        w_sb = {}

        def load_weight_chunk(name, j, eng=None):
            ts = w_sb.get(name)
            if ts is None:
                ts = [w_pool.tile([128, HT * 128], FP8, tag=f"{name}{g}",
                                  name=f"w_{name}{g}") for g in range(HT)]
                w_sb[name] = ts
            step = HT * 128
            (eng or nc.sync).dma_start(ts[j][:],
                                       w_ext[name][:, j * step:(j + 1) * step])

        def wpair(name, ot, j):
            """lhsT [128, 2, 128]: out-tile ot, contraction h-tile pair (2j, 2j+1)."""
            c0 = 2 * j * 128
            return (w_sb[name][ot][:, c0: c0 + 256]
                    .rearrange("p (t m) -> p t m", t=2))

        # ---- stage-1 DMAs in first-use order (Sync queue) ----
        def load_x_chunks(ext, nb, tagbase):
            """One projection input block as 3 ht-pair tiles [128, 2*QB]."""
            tiles = []
            for j in range(3):
                t = in_pool.tile([128, 2 * QB], FP8, tag=f"{tagbase}{j}",
                                 name=f"x_{tagbase}{j}_{nb}")
                c0 = nb * HT * QB + j * 2 * QB
                nc.sync.dma_start(t[:], ext[:, c0: c0 + 2 * QB])
                tiles.append(t)
            return tiles

        load_weight_chunk("wkT", 0)
        kx0 = []
        for j in range(3):
            t = in_pool.tile([128, 2 * QB], FP8, tag=f"kx{j}", name=f"x_kx{j}_0")
            nc.sync.dma_start(t[:], kT_ext[:, j * 2 * QB:(j + 1) * 2 * QB])
            kx0.append(t)
        for j in range(1, HT):
            load_weight_chunk("wkT", j, eng=nc.scalar)

        bias_sb = const_pool.tile([128, 3 * HT], dt.float32, tag="biases")
        nc.sync.dma_start(bias_sb[:], b_ext[:])
        id64_sb = const_pool.tile([128, 128], dt.bfloat16, tag="ident64")
        nc.scalar.dma_start(id64_sb[:], id64_ext[:])
        biases = {name: bias_sb[:, i * HT:(i + 1) * HT]
                  for i, name in enumerate(("bq", "bo1", "bo2"))}

        # ---- activation tiles ----
        KT = [act_pool.tile([128, HT * QB], FP8, tag=f"KT{nb}",
                            name=f"KT{nb}") for nb in range(4)]
        KT3 = [t[:].rearrange("p (t k) -> p t k", t=HT) for t in KT]
        QT = [[act_pool.tile([128, 2 * QB], FP8, tag=f"QT{qb}_{j}",
                             name=f"QT{qb}_{j}") for j in range(3)]
              for qb in range(NQB)]

        ones_f32 = const_pool.tile([128, 128], dt.float32, tag="ones_f32")
        nc.vector.memset(ones_f32[:], 1.0)
        ones_row = const_pool.tile([1, 128], dt.float32r, tag="ones_row")
        nc.vector.tensor_copy(ones_row[:], ones_f32[0:1, :])
        ones_col = const_pool.tile([128, 1], dt.float32r, tag="ones_col")
        nc.vector.tensor_copy(ones_col[:], ones_f32[:, 0:1])

        # 8-bank PSUM rotation for the projection phase (PV banks are free).
        _rot = {"i": 0}

        def proj_ps(nm):
            i = _rot["i"]
            _rot["i"] += 1
            if i % 8 < 2:
                return ps_gen.tile([128, QB], dt.float32, tag="gen", name=nm)
            return ps_pool.tile([128, QB], dt.float32, tag=f"ps_w{i % 8 - 2}",
                                name=nm)

        def proj_tile(wname, xt, ot, ps):
            """3 DoubleRow matmuls: one [128out, QB] projection tile."""
            for j in range(3):
                nc.tensor.matmul(
                    ps[:], wpair(wname, ot, j),
                    xt[j][:].rearrange("p (t q) -> p t q", t=2),
                    start=(j == 0), stop=(j == 2), perf_mode=PMODE,
                )

        # ---- K projection: 24 tiles, DVE evictions (no bias - bk drops) ----
        k_tile_fns = []

        def k_tile(nb, xt, ot):
            ps = ps_gen.tile([128, QB], dt.float32, tag="gen",
                             name=f"ps_k3_{ot}")
            proj_tile("wkT", xt, ot, ps)
            nc.vector.tensor_scalar_mul(KT[nb][:, ot * QB:(ot + 1) * QB],
                                        ps[:], 1.0 / WSCALE)

        cur = kx0
        for nb in range(4):
            nxt = load_x_chunks(kT_ext, nb + 1, "kx") if nb < 3 else None
            if nb == 1:
                v_blks = []
                for c in range(4):
                    t = act_pool.tile([128, 4 * H], FP8, tag=f"v_in{c}",
                                      name=f"v_in{c}")
                    nc.scalar.dma_start(t[:], v_ext[:, c * 4 * H:(c + 1) * 4 * H])
                    v_blks.append(t)
            elif nb == 2:
                pass
            for ot in range(HT):
                ps = proj_ps(f"ps_k_{nb}_{ot}")
                proj_tile("wkT", cur, ot, ps)
                dst = KT[nb][:, ot * QB:(ot + 1) * QB]
                # all evictions on DVE: anything on the Scalar stream lands
                # behind DIRECT2D descriptor-generation bursts and stalls
                # the PSUM rotation
                nc.vector.tensor_scalar_mul(dst, ps[:], 1.0 / WSCALE)
            cur = nxt

        def vpair(jk, ht):
            """lhsT [128, 2, 128]: k-tile pair (2jk, 2jk+1), h-tile ht."""
            t = v_blks[jk // 2]
            j2 = (jk % 2) * 2
            return (t[:].rearrange("p (t h) -> p t h", t=4)
                    [:, j2: j2 + 2, ht * 128:(ht + 1) * 128])

        # ---- Q projection nb=0 (ScalarE evictions apply bias + 1/64) ----
        for j in range(HT):
            load_weight_chunk("wqT", j)
        qx0 = load_x_chunks(qT_ext, 0, "qx")
        qx1 = load_x_chunks(qT_ext, 1, "qx")
        for j in range(HT):
            load_weight_chunk("wo1T", j)
        for j in range(HT):
            load_weight_chunk("wo2T", j)

        def q_tile(qb, xt, ot, gen_only=False):
            # interleaved tiles (inside attention) must not touch the live
            # PV accumulator banks - gen rotation only
            if gen_only:
                ps = ps_gen.tile([128, QB], dt.float32, tag="gen",
                                 name=f"ps_q_{qb}_{ot}")
            else:
                ps = proj_ps(f"ps_q_{qb}_{ot}")
            proj_tile("wqT", xt, ot, ps)
            nc.scalar.activation(
                QT[qb][ot // 2][:, (ot % 2) * QB:(ot % 2 + 1) * QB], ps[:],
                AF.Identity,
                bias=biases["bq"][:, ot: ot + 1], scale=1.0 / WSCALE,
            )

        # ---- vT (bf16 residual) as ht-pair tiles ----
        def load_vt(qb, eng):
            tiles = []
            for j in range(3):
                t = vt_pool.tile([128, 2 * QB], dt.float32, tag=f"vT{j}",
                                 name=f"vT{j}_{qb}")
                c0 = qb * HT * QB + j * 2 * QB
                eng.dma_start(t[:], vT_ext[:, c0: c0 + 2 * QB])
                tiles.append(t)
            return tiles

        state = {}
        state[0] = {"vT": load_vt(0, nc.sync)}
        for ot in range(HT):
            q_tile(0, qx0, ot)

        def attn_block(qb, filler):
            """Merged scores+PV for q-block qb; filler(slot) emits interleaved
            PE work (proj/MLP tiles) - called with slot index 0..7 per jk."""
            st = state.setdefault(qb, {})
            if "vT" not in st:
                st["vT"] = load_vt(qb, nc.sync)
            sum_a = st_pool.tile([128, QB], dt.float32r, tag="sum_a",
                                 name=f"sum_a{qb}")
            sum_b = st_pool.tile([128, QB], dt.float32r, tag="sum_b",
                                 name=f"sum_b{qb}")
            ps_w = [ps_pool.tile([128, QB], dt.float32, tag=f"ps_w{ht}",
                                 name=f"ps_w{ht}_{qb}")
                    for ht in range(HT)]
            exp_pairs = []
            for jk in range(KTILES // 2):
                pair = exp_pool.tile([128, 2 * QB], FP8, tag="expS",
                                     name=f"expS_{qb}_{jk}")
                exp_pairs.append(pair)
                for t2 in range(2):
                    kt = 2 * jk + t2
                    ps_s = ps_gen.tile([128, QB], dt.float32, tag="gen",
                                       name=f"ps_s_{qb}_{kt}")
                    for jo in range(3):
                        nc.tensor.matmul(
                            ps_s[:],
                            KT3[kt // 4][:, 2 * jo: 2 * jo + 2,
                                         (kt % 4) * 128:(kt % 4 + 1) * 128],
                            QT[qb][jo][:].rearrange("p (t q) -> p t q", t=2),
                            start=(jo == 0), stop=(jo == 2), perf_mode=PMODE,
                        )
                    half = pair[:, t2 * QB:(t2 + 1) * QB]
                    nc.scalar.activation(half, ps_s[:], AF.Exp,
                                         scale=1.0 / QSCALE)
                    # two DVE accumulators (GpSimd fp8 2-input ops measured
                    # far too slow); two chains halve the serial dependency
                    if kt == 0:
                        nc.vector.tensor_copy(sum_b[:], half)
                    elif kt == 1:
                        nc.vector.tensor_copy(sum_a[:], half)
                    elif kt % 2 == 0:
                        nc.vector.tensor_add(sum_b[:], sum_b[:], half)
                    else:
                        nc.vector.tensor_add(sum_a[:], sum_a[:], half)
                filler(qb, jk)
                if jk >= 1:
                    _pv_group(qb, jk - 1, exp_pairs[jk - 1], ps_w,
                              start=(jk == 1), stop=False)
            _pv_group(qb, 7, exp_pairs[7], ps_w, start=False, stop=True)
            st["ps_w"] = ps_w

            # rowsum (both accumulators) -> 1/x on [1,512] -> PE broadcast
            ps_sum = ps_gen.tile([1, QB], dt.float32, tag="gen",
                                 name=f"ps_sum{qb}")
            nc.tensor.matmul(ps_sum[:], ones_col[:], sum_a[:],
                             start=True, stop=False)
            nc.tensor.matmul(ps_sum[:], ones_col[:], sum_b[:],
                             start=False, stop=True)
            # 1/x as exp(-ln(x)) on ScalarE: DVE reciprocal costs 3.4us even
            # on [1,512] (per-lane serial), ScalarE runs [1,512] in ~0.7us/op
            # and can emit float32r for the broadcast matmul directly
            lnd = st_pool.tile([1, QB], dt.float32, tag="lnd",
                               name=f"lnd{qb}")
            nc.scalar.activation(lnd[:], ps_sum[:], AF.Ln)
            rsum_r = st_pool.tile([1, QB], dt.float32r, tag="rsum",
                                  name=f"rsum{qb}")
            nc.scalar.activation(rsum_r[:], lnd[:], AF.Exp, scale=-1.0)
            filler(qb, 8)   # PE work while ScalarE runs ln/exp
            ps_b = ps_gen.tile([128, QB], dt.float32, tag="gen",
                               name=f"ps_b{qb}")
            nc.tensor.matmul(ps_b[:], ones_row[:], rsum_r[:],
                             start=True, stop=True)
            bcast = st_pool.tile([128, QB], dt.float32, tag="bcast",
                                 name=f"bcast{qb}")
            nc.scalar.copy(bcast[:], ps_b[:])
            filler(qb, 9)   # PE work while the broadcast is copied out
            st["bcast"] = bcast

        def _pv_group(qb, jk, pair, ps_w, start, stop):
            rhs = pair[:].rearrange("p (t q) -> p t q", t=2)
            for ht in range(HT):
                nc.tensor.matmul(ps_w[ht][:], vpair(jk, ht), rhs,
                                 start=start, stop=stop, perf_mode=PMODE)

        def weighted(qb):
            """weighted^T = PV * bcast + value^T; bf16 store + fp8 copy."""
            st = state[qb]
            wT_bf, wT_f8 = [], []
            for j in range(3):
                wT_bf.append(wt_pool.tile([128, 2 * QB], dt.bfloat16,
                                          tag=f"wTb{j}", name=f"wTb{j}_{qb}"))
                wT_f8.append(wt_pool.tile([128, 2 * QB], FP8,
                                          tag=f"wT8{j}", name=f"wT8{j}_{qb}"))
            for ht in range(HT):
                j, h2 = divmod(ht, 2)
                tmp = tmp_pool.tile([128, QB], dt.float32, tag="wtmp",
                                    name=f"wtmp_{qb}_{ht}")
                nc.vector.tensor_mul(tmp[:], st["ps_w"][ht][:], st["bcast"][:])
                nc.vector.tensor_add(
                    wT_bf[j][:, h2 * QB:(h2 + 1) * QB], tmp[:],
                    st["vT"][j][:, h2 * QB:(h2 + 1) * QB],
                )
                if h2 == 1:
                    nc.vector.tensor_copy(wT_f8[j][:], wT_bf[j][:])
            st["wT_bf"] = wT_bf
            st["wT_f8"] = wT_f8

        def mlp_h1_tile(qb, ot, rot=False):
            st = state[qb]
            if "h1" not in st:
                st["h1"] = [h1_pool.tile([128, 2 * QB], FP8, tag=f"h1_{j}",
                                         name=f"h1_{j}_{qb}")
                            for j in range(3)]
            if rot:
                ps = proj_ps(f"ps_h1_{qb}_{ot}")
            else:
                ps = ps_gen.tile([128, QB], dt.float32, tag="gen",
                                 name=f"ps_h1_{qb}_{ot}")
            for j in range(3):
                nc.tensor.matmul(
                    ps[:], wpair("wo1T", ot, j),
                    st["wT_f8"][j][:].rearrange("p (t q) -> p t q", t=2),
                    start=(j == 0), stop=(j == 2), perf_mode=PMODE,
                )
            j, h2 = divmod(ot, 2)
            nc.scalar.activation(
                st["h1"][j][:, h2 * QB:(h2 + 1) * QB], ps[:], AF.Relu,
                bias=biases["bo1"][:, ot: ot + 1], scale=1.0 / WSCALE,
            )

        def mlp_out_tile(qb, ot, rot=False):
            st = state[qb]
            if rot:
                ps = proj_ps(f"ps_o_{qb}_{ot}")
            else:
                ps = ps_gen.tile([128, QB], dt.float32, tag="gen",
                                 name=f"ps_o_{qb}_{ot}")
            for j in range(3):
                nc.tensor.matmul(
                    ps[:], wpair("wo2T", ot, j),
                    st["h1"][j][:].rearrange("p (t q) -> p t q", t=2),
                    start=(j == 0), stop=False, perf_mode=PMODE,
                )
            # residual rides the PE: PSUM += 64*I @ weighted^T (bf16, exact),
            # so the eviction is a single ScalarE op (scale 1/64 + bias) and
            # no DVE add sits on the output critical path
            j, h2 = divmod(ot, 2)
            nc.tensor.matmul(
                ps[:], id64_sb[:],
                st["wT_bf"][j][:, h2 * QB:(h2 + 1) * QB],
                start=False, stop=True,
            )
            o_sb = o_pool.tile([128, QB], dt.float32, tag="outT_blk",
                               name=f"outT_{qb}_{ot}")
            if ot % 2 == 0:
                nc.scalar.activation(o_sb[:], ps[:], AF.Identity,
                                     bias=biases["bo2"][:, ot: ot + 1],
                                     scale=1.0 / WSCALE)
            else:
                nc.vector.tensor_scalar(o_sb[:], ps[:], 1.0 / WSCALE,
                                        biases["bo2"][:, ot: ot + 1],
                                        mybir.AluOpType.mult,
                                        mybir.AluOpType.add)
            eng = nc.sync if ot % 2 == 0 else nc.scalar
            eng.dma_start(
                outT_ext[:, (qb * HT + ot) * QB:(qb * HT + ot + 1) * QB],
                o_sb[:],
            )

        # ---- attention block 0: interleave Q-proj nb=1 tiles; slots 8/9
        # bracket the softmax normalization chain with PE work ----
        def fill0(qb, jk):
            if jk in (5, 6, 7):
                q_tile(1, qx1, jk - 5, gen_only=True)
            elif jk == 8:
                q_tile(1, qx1, 3, gen_only=True)
                q_tile(1, qx1, 4, gen_only=True)
            elif jk == 9:
                q_tile(1, qx1, 5, gen_only=True)

        attn_block(0, fill0)
        weighted(0)

        # ---- attention block 1: interleave MLP(0) h1 tiles ----
        def fill1(qb, jk):
            if jk in (5, 6, 7):
                mlp_h1_tile(0, jk - 5)
            elif jk == 8:
                mlp_h1_tile(0, 3)
                mlp_h1_tile(0, 4)
            elif jk == 9:
                mlp_h1_tile(0, 5)

        attn_block(1, fill1)
        # weighted(1) first: its DVE ops are the critical path into MLP(1);
        # mlp_out(0) and mlp_h1(1) alternate underneath it so the PE never
        # waits on a single eviction stream
        weighted(1)
        # the PE is in-order: emit all dependency-free mlp_out(0) tiles
        # before the first mlp_h1(1) tile (which waits for weighted(1)'s
        # first pair + fp8 cast). The MLP(1) tail rotates through the freed
        # PV PSUM banks so evictions never stall the 2-bank gen rotation.
        for ot in range(HT):
            mlp_out_tile(0, ot)
        for ot in range(HT):
            mlp_h1_tile(1, ot, rot=True)
        for ot in range(HT):
            mlp_out_tile(1, ot, rot=True)


# ---- host-side shard packing ----

def _tile_rows(a):
    """[T*128, N] -> [128, T*N]: partition-tiled T-layout, contiguous DMA."""
    t = a.shape[0] // 128
    return a.reshape(t, 128, a.shape[1]).transpose(1, 0, 2).reshape(128, -1)


def _tile_weight(w):
    """W^T [768h, 768o] -> [128, (ot, ht, 128)]: o-major packed lhsT tiles."""
    x = w.reshape(HT, 128, HT, 128)          # [ht, p, ot, o128]
    return x.transpose(1, 2, 0, 3).reshape(128, -1)


def _tile_rows_blocked(a, qb):
    """[768, NB*qb] -> [128, NB*(6*qb)]: per-block ht-major packing."""
    nb = a.shape[1] // qb
    x = a.reshape(HT, 128, nb, qb).transpose(1, 2, 0, 3)
    return x.reshape(128, -1)


def shard_inputs(query, key, value, Wq, bq, Wk, bk, Wo1, bo1, Wo2, bo2):
    """Full inputs -> per-core in_maps (host packing, fp8 cast, scale folds)."""
    scale = np.float32(1.0 / np.sqrt(np.float32(H)))

    def c8(x):
        return np.ascontiguousarray(
            np.clip(np.asarray(x, np.float32), -240, 240).astype(NP_FP8))

    def cb(x):
        return np.ascontiguousarray(np.asarray(x, np.float32).astype(NP_BF))

    def cf(x):
        return np.ascontiguousarray(x.astype(np.float32))

    shared = {
        "ident64": np.ascontiguousarray((np.eye(128, dtype=np.float32)
                                         * WSCALE).astype(NP_BF)),
        "wqT": c8(_tile_weight(Wq.T * (scale * QSCALE * WSCALE))),
        "wkT": c8(_tile_weight(Wk.T * WSCALE)),
        "wo1T": c8(_tile_weight(Wo1.T * WSCALE)),
        "wo2T": c8(_tile_weight(Wo2.T * WSCALE)),
        "biases": cf(np.concatenate([
            (bq * scale * QSCALE).reshape(HT, 128).T,
            bo1.reshape(HT, 128).T,
            bo2.reshape(HT, 128).T], axis=1)),
    }
    in_maps = []
    for core in range(N_CORES):
        b, half = divmod(core, 2)
        r0 = half * QCHUNK
        in_maps.append({
            "qT": c8(_tile_rows_blocked(query[b].T[:, r0: r0 + QCHUNK], QB)),
            "kT": c8(_tile_rows_blocked(key[b].T, QB)),
            "v": c8(_tile_rows(value[b])),
            "vT": cf(_tile_rows_blocked(value[b].T[:, r0: r0 + QCHUNK], QB)),
            **shared,
        })
    return in_maps


def gather_outputs(results):
    """Per-core outT [128, NQB*HT*QB] -> full [B, S, H]."""
    out = np.empty((B, S, H), dtype=np.float32)
    for core in range(N_CORES):
        b, half = divmod(core, 2)
        r0 = half * QCHUNK
        buf = results[core]["outT"].reshape(128, NQB, HT, QB)
        # out[q0+qb*QB+n, ot*128+p] = buf[p, qb, ot, n]
        out[b, r0: r0 + QCHUNK] = (
            buf.transpose(1, 3, 2, 0).reshape(QCHUNK, H)
        )
    return out


def run(inputs, trace=False):
    nc = build_kernel()
    in_maps = shard_inputs(**{k: np.asarray(v) for k, v in inputs.items()})
    res = run_bass_kernel_spmd(nc, in_maps, list(range(N_CORES)), trace=trace)
    return gather_outputs(res.results), res


def _split_multi_waits(nc):
    """Workaround for this container's walrus rejecting instructions that
    carry more than one semaphore wait ("Too many sync wait commands"):
    hoist N-1 waits onto fresh single-wait same-engine InstNoOp instructions
    inserted immediately before the instruction. Engine streams execute the
    block's per-engine subsequence in order, so blocking on the nops first is
    semantically identical to one multi-wait instruction."""
    for f in nc.m.functions:
        for bb in f.blocks:
            insts = list(bb.instructions)
            out = []
            changed = False
            for inst in insts:
                si = inst.sync_info
                waits = list(si.on_wait) if si is not None and si.on_wait else []
                if len(waits) > 1:
                    changed = True
                    for w in waits[:-1]:
                        nop = mybir.InstNoOp(
                            name=nc.get_next_instruction_name(), ins=[], outs=[]
                        )
                        nop.engine = inst.engine
                        nop.sync_info = mybir.SyncInfo(on_wait=[w], on_update=[])
                        out.append(nop)
                    si.on_wait = waits[-1:]
                    inst.sync_info = si
                out.append(inst)
            if changed:
                bb.instructions = out


def kernel(**inputs):
    """Entry point: full (unsharded) numpy inputs -> full [B, S, H] output."""
    out, _ = run(inputs, trace=False)
    return out


# revision 32
# speedup vs baseline: 1.1196x; 1.0155x over previous
"""Distributed single-head attention + MLP block for 8 TRN2 NeuronCores.

Reference computation (per batch b):
  Q = query @ Wq^T + bq ; K = key @ Wk^T + bk
  scores = Q @ K^T / sqrt(H) ; attn = softmax(scores)
  weighted = attn @ value + value
  h1 = relu(weighted @ Wo1^T + bo1)
  out = h1 @ Wo2^T + bo2 + weighted

Sharding: B=4 batches x 2 query-row halves = 8 shards. Each core gets its
1024 query rows plus the full 2048 keys/values of its batch; attention is
dense (non-causal) so no inter-core communication is needed.

Layout: everything on-device is feature-on-partitions ("T-layout",
X^T[f, tok]) so all matmul contractions line up with zero on-device
transposes; the host pre-packs every shard into the exact [128, free]
SBUF tiling the kernel consumes. All matmuls run fp8e4 DoubleRow (2x).

Numerics tricks (all folded on host / into activation scale operands):
  - softmax shift-invariance: K bias bk drops entirely (k-independent
    terms cancel; bq is kept on the Q side).
  - Q is stored as 32x true-Q and K as true-K so both live in fp8's
    normal range (raw Q*scale has std ~0.02 - subnormal); the scores
    PSUM is then 32x the true logits and the exp activation applies
    scale=1/32.
  - fp8 weights are stored 64x and unscaled via activation scale=1/64
    (uniform(-0.036, 0.036) weights would otherwise be ~half subnormal).
  - softmax needs no max-subtraction: logits have std ~1/3.

Schedule (PE never idles between phases):
  K-proj (24 tiles over an 8-bank PSUM rotation, DVE evictions)
  Q-proj nb=0, then attention block 0 with Q-proj nb=1 tiles interleaved
  attention block 1 with MLP(0) tiles interleaved, then MLP(1).
Attention merges scores and PV per k-tile pair: 3 score MMs -> exp on
ScalarE -> 6 PV MMs one pair behind, so PSUM-evict latency is always
covered by other matmuls. The softmax denominator accumulates on DVE
(even k-tiles) and GpSimd (odd k-tiles); the rowsum matmul sums both,
reciprocal runs on [1,512] only, and the PE broadcasts it back.
Bulk prefetches (v, vT, MLP weights) issue on the Scalar HWDGE queue
during the K-proj phase (ScalarE is idle there); everything else on Sync.
"""

import contextlib

import numpy as np
import ml_dtypes

import concourse.bass as bass
import concourse.mybir as mybir
import concourse.tile as tile
from concourse.bass_utils import run_bass_kernel_spmd

dt = mybir.dt
AF = mybir.ActivationFunctionType

H = 768          # model dim
B = 4            # batch
S = 2048         # sequence length
N_CORES = 8
QCHUNK = S * B // N_CORES        # 1024 query rows per core
HT = H // 128                    # 6 feature partition-tiles
KTILES = S // 128                # 16 key partition-tiles
QB = 512                         # q-block width (= PSUM bank, fp32)
NQB = QCHUNK // QB               # 2 q-blocks per core

FP8 = dt.float8e4
NP_FP8 = dt.np(FP8)
NP_BF = ml_dtypes.bfloat16
PMODE = mybir.MatmulPerfMode.DoubleRow

WSCALE = 64.0      # fp8 weight pre-scale (host) -> activation scale 1/64
QSCALE = 32.0      # stored Q = 32x true Q -> exp scale 1/32


def build_kernel():
    nc = bass.Bass()

    qT_ext = nc.declare_dram_parameter("qT", [128, NQB * HT * QB], FP8, isOutput=False)
    kT_ext = nc.declare_dram_parameter("kT", [128, (S // QB) * HT * QB], FP8, isOutput=False)
    v_ext = nc.declare_dram_parameter("v", [128, KTILES * H], FP8, isOutput=False)
    vT_ext = nc.declare_dram_parameter("vT", [128, NQB * HT * QB], dt.float32, isOutput=False)
    w_ext = {
        name: nc.declare_dram_parameter(name, [128, HT * H], FP8, isOutput=False)
        for name in ("wqT", "wkT", "wo1T", "wo2T")
    }
    b_ext = nc.declare_dram_parameter("biases", [128, 3 * HT], dt.float32,
                                      isOutput=False)
    id64_ext = nc.declare_dram_parameter("ident64", [128, 128], dt.bfloat16,
                                         isOutput=False)
    outT_ext = nc.declare_dram_parameter(
        "outT", [128, NQB * HT * QB], dt.float32, isOutput=True
    )

    with tile.TileContext(nc) as tc, nc.allow_low_precision(
        reason="fp8 matmul path is intentional; rel-err budget is 2e-2"
    ):
        _body(nc, tc, qT_ext, kT_ext, v_ext, vT_ext, w_ext, b_ext, id64_ext,
              outT_ext)

    _split_multi_waits(nc)
    return nc


def _body(nc, tc, qT_ext, kT_ext, v_ext, vT_ext, w_ext, b_ext, id64_ext,
          outT_ext):
    with contextlib.ExitStack() as ctx:
        const_pool = ctx.enter_context(tc.tile_pool(name="const", bufs=1))
        w_pool = ctx.enter_context(tc.tile_pool(name="w", bufs=1))
        act_pool = ctx.enter_context(tc.tile_pool(name="act", bufs=1))
        in_pool = ctx.enter_context(tc.tile_pool(name="inp", bufs=2))
        vt_pool = ctx.enter_context(tc.tile_pool(name="vt", bufs=2))
        wt_pool = ctx.enter_context(tc.tile_pool(name="wt", bufs=2))
        h1_pool = ctx.enter_context(tc.tile_pool(name="h1", bufs=2))
        st_pool = ctx.enter_context(tc.tile_pool(name="st", bufs=2))
        tmp_pool = ctx.enter_context(tc.tile_pool(name="tmp", bufs=4))
        o_pool = ctx.enter_context(tc.tile_pool(name="o", bufs=6))
        exp_pool = ctx.enter_context(tc.tile_pool(name="exps", bufs=6))
        # PSUM: 6 PV accumulators + 2 general banks = 8 banks exactly.
        ps_pool = ctx.enter_context(tc.tile_pool(name="ps", bufs=1, space="PSUM"))
        ps_gen = ctx.enter_context(tc.tile_pool(name="ps_gen", bufs=2, space="PSUM"))

        # ---- weight / bias loading helpers ----
        w_sb = {}

        def load_weight_chunk(name, j, eng=None):
            ts = w_sb.get(name)
            if ts is None:
                ts = [w_pool.tile([128, HT * 128], FP8, tag=f"{name}{g}",
                                  name=f"w_{name}{g}") for g in range(HT)]
                w_sb[name] = ts
            step = HT * 128
            (eng or nc.sync).dma_start(ts[j][:],
                                       w_ext[name][:, j * step:(j + 1) * step])

        def wpair(name, ot, j):
            """lhsT [128, 2, 128]: out-tile ot, contraction h-tile pair (2j, 2j+1)."""
            c0 = 2 * j * 128
            return (w_sb[name][ot][:, c0: c0 + 256]
                    .rearrange("p (t m) -> p t m", t=2))

        # ---- stage-1 DMAs in first-use order (Sync queue) ----
        def load_x_chunks(ext, nb, tagbase):
            """One projection input block as 3 ht-pair tiles [128, 2*QB]."""
            tiles = []
            for j in range(3):
                t = in_pool.tile([128, 2 * QB], FP8, tag=f"{tagbase}{j}",
                                 name=f"x_{tagbase}{j}_{nb}")
                c0 = nb * HT * QB + j * 2 * QB
                nc.sync.dma_start(t[:], ext[:, c0: c0 + 2 * QB])
                tiles.append(t)
            return tiles

        load_weight_chunk("wkT", 0)
        kx0 = []
        for j in range(3):
            t = in_pool.tile([128, 2 * QB], FP8, tag=f"kx{j}", name=f"x_kx{j}_0")
            nc.sync.dma_start(t[:], kT_ext[:, j * 2 * QB:(j + 1) * 2 * QB])
            kx0.append(t)
        for j in range(1, HT):
            load_weight_chunk("wkT", j, eng=nc.scalar)

        bias_sb = const_pool.tile([128, 3 * HT], dt.float32, tag="biases")
        nc.sync.dma_start(bias_sb[:], b_ext[:])
        id64_sb = const_pool.tile([128, 128], dt.bfloat16, tag="ident64")
        nc.scalar.dma_start(id64_sb[:], id64_ext[:])
        biases = {name: bias_sb[:, i * HT:(i + 1) * HT]
                  for i, name in enumerate(("bq", "bo1", "bo2"))}

        # ---- activation tiles ----
        KT = [act_pool.tile([128, HT * QB], FP8, tag=f"KT{nb}",
                            name=f"KT{nb}") for nb in range(4)]
        KT3 = [t[:].rearrange("p (t k) -> p t k", t=HT) for t in KT]
        QT = [[act_pool.tile([128, 2 * QB], FP8, tag=f"QT{qb}_{j}",
                             name=f"QT{qb}_{j}") for j in range(3)]
              for qb in range(NQB)]

        ones_f32 = const_pool.tile([128, 128], dt.float32, tag="ones_f32")
        nc.vector.memset(ones_f32[:], 1.0)
        ones_row = const_pool.tile([1, 128], dt.float32r, tag="ones_row")
        nc.vector.tensor_copy(ones_row[:], ones_f32[0:1, :])
        ones_col = const_pool.tile([128, 1], dt.float32r, tag="ones_col")
        nc.vector.tensor_copy(ones_col[:], ones_f32[:, 0:1])

        # 8-bank PSUM rotation for the projection phase (PV banks are free).
        _rot = {"i": 0}

        def proj_ps(nm):
            i = _rot["i"]
            _rot["i"] += 1
            if i % 8 < 2:
                return ps_gen.tile([128, QB], dt.float32, tag="gen", name=nm)
            return ps_pool.tile([128, QB], dt.float32, tag=f"ps_w{i % 8 - 2}",
                                name=nm)

        def proj_tile(wname, xt, ot, ps):
            """3 DoubleRow matmuls: one [128out, QB] projection tile."""
            for j in range(3):
                nc.tensor.matmul(
                    ps[:], wpair(wname, ot, j),
                    xt[j][:].rearrange("p (t q) -> p t q", t=2),
                    start=(j == 0), stop=(j == 2), perf_mode=PMODE,
                )

        # ---- K projection: 24 tiles, DVE evictions (no bias - bk drops) ----
        k_tile_fns = []

        def k_tile(nb, xt, ot):
            ps = ps_gen.tile([128, QB], dt.float32, tag="gen",
                             name=f"ps_k3_{ot}")
            proj_tile("wkT", xt, ot, ps)
            nc.vector.tensor_scalar_mul(KT[nb][:, ot * QB:(ot + 1) * QB],
                                        ps[:], 1.0 / WSCALE)

        cur = kx0
        for nb in range(4):
            nxt = load_x_chunks(kT_ext, nb + 1, "kx") if nb < 3 else None
            if nb == 1:
                v_blks = []
                for c in range(4):
                    t = act_pool.tile([128, 4 * H], FP8, tag=f"v_in{c}",
                                      name=f"v_in{c}")
                    nc.scalar.dma_start(t[:], v_ext[:, c * 4 * H:(c + 1) * 4 * H])
                    v_blks.append(t)
            elif nb == 2:
                pass
            for ot in range(HT):
                ps = proj_ps(f"ps_k_{nb}_{ot}")
                proj_tile("wkT", cur, ot, ps)
                dst = KT[nb][:, ot * QB:(ot + 1) * QB]
                # all evictions on DVE: anything on the Scalar stream lands
                # behind DIRECT2D descriptor-generation bursts and stalls
                # the PSUM rotation
                nc.vector.tensor_scalar_mul(dst, ps[:], 1.0 / WSCALE)
            cur = nxt

        def vpair(jk, ht):
            """lhsT [128, 2, 128]: k-tile pair (2jk, 2jk+1), h-tile ht."""
            t = v_blks[jk // 2]
            j2 = (jk % 2) * 2
            return (t[:].rearrange("p (t h) -> p t h", t=4)
                    [:, j2: j2 + 2, ht * 128:(ht + 1) * 128])

        # ---- Q projection nb=0 (ScalarE evictions apply bias + 1/64) ----
        for j in range(HT):
            load_weight_chunk("wqT", j)
        qx0 = load_x_chunks(qT_ext, 0, "qx")
        qx1 = load_x_chunks(qT_ext, 1, "qx")
        for j in range(HT):
            load_weight_chunk("wo1T", j)
        for j in range(HT):
            load_weight_chunk("wo2T", j)

        def q_tile(qb, xt, ot, gen_only=False):
            # interleaved tiles (inside attention) must not touch the live
            # PV accumulator banks - gen rotation only
            if gen_only:
                ps = ps_gen.tile([128, QB], dt.float32, tag="gen",
                                 name=f"ps_q_{qb}_{ot}")
            else:
                ps = proj_ps(f"ps_q_{qb}_{ot}")
            proj_tile("wqT", xt, ot, ps)
            dst = QT[qb][ot // 2][:, (ot % 2) * QB:(ot % 2 + 1) * QB]
            if gen_only:
                # inside attention ScalarE is busy with exps - evict on DVE
                nc.vector.tensor_scalar(dst, ps[:], 1.0 / WSCALE,
                                        biases["bq"][:, ot: ot + 1],
                                        mybir.AluOpType.mult,
                                        mybir.AluOpType.add)
            else:
                nc.scalar.activation(dst, ps[:], AF.Identity,
                                     bias=biases["bq"][:, ot: ot + 1],
                                     scale=1.0 / WSCALE)

        # ---- vT (bf16 residual) as ht-pair tiles ----
        def load_vt(qb, eng):
            tiles = []
            for j in range(3):
                t = vt_pool.tile([128, 2 * QB], dt.float32, tag=f"vT{j}",
                                 name=f"vT{j}_{qb}")
                c0 = qb * HT * QB + j * 2 * QB
                eng.dma_start(t[:], vT_ext[:, c0: c0 + 2 * QB])
                tiles.append(t)
            return tiles

        state = {}
        state[0] = {"vT": load_vt(0, nc.sync)}
        for ot in range(HT):
            q_tile(0, qx0, ot)

        def attn_block(qb, filler):
            """Merged scores+PV for q-block qb; filler(slot) emits interleaved
            PE work (proj/MLP tiles) - called with slot index 0..7 per jk."""
            st = state.setdefault(qb, {})
            if "vT" not in st:
                st["vT"] = load_vt(qb, nc.sync)
            sum_a = st_pool.tile([128, QB], dt.float32r, tag="sum_a",
                                 name=f"sum_a{qb}")
            sum_b = st_pool.tile([128, QB], dt.float32r, tag="sum_b",
                                 name=f"sum_b{qb}")
            ps_w = [ps_pool.tile([128, QB], dt.float32, tag=f"ps_w{ht}",
                                 name=f"ps_w{ht}_{qb}")
                    for ht in range(HT)]
            exp_pairs = []
            for jk in range(KTILES // 2):
                pair = exp_pool.tile([128, 2 * QB], FP8, tag="expS",
                                     name=f"expS_{qb}_{jk}")
                exp_pairs.append(pair)
                for t2 in range(2):
                    kt = 2 * jk + t2
                    ps_s = ps_gen.tile([128, QB], dt.float32, tag="gen",
                                       name=f"ps_s_{qb}_{kt}")
                    for jo in range(3):
                        nc.tensor.matmul(
                            ps_s[:],
                            KT3[kt // 4][:, 2 * jo: 2 * jo + 2,
                                         (kt % 4) * 128:(kt % 4 + 1) * 128],
                            QT[qb][jo][:].rearrange("p (t q) -> p t q", t=2),
                            start=(jo == 0), stop=(jo == 2), perf_mode=PMODE,
                        )
                    half = pair[:, t2 * QB:(t2 + 1) * QB]
                    nc.scalar.activation(half, ps_s[:], AF.Exp,
                                         scale=1.0 / QSCALE)
                    # two DVE accumulators (GpSimd fp8 2-input ops measured
                    # far too slow); two chains halve the serial dependency
                    if kt == 0:
                        nc.vector.tensor_copy(sum_b[:], half)
                    elif kt == 1:
                        nc.vector.tensor_copy(sum_a[:], half)
                    elif kt % 2 == 0:
                        nc.vector.tensor_add(sum_b[:], sum_b[:], half)
                    else:
                        nc.vector.tensor_add(sum_a[:], sum_a[:], half)
                filler(qb, jk)
                if jk >= 1:
                    _pv_group(qb, jk - 1, exp_pairs[jk - 1], ps_w,
                              start=(jk == 1), stop=False)
            _pv_group(qb, 7, exp_pairs[7], ps_w, start=False, stop=True)
            st["ps_w"] = ps_w

            # rowsum (both accumulators) -> 1/x on [1,512] -> PE broadcast
            ps_sum = ps_gen.tile([1, QB], dt.float32, tag="gen",
                                 name=f"ps_sum{qb}")
            nc.tensor.matmul(ps_sum[:], ones_col[:], sum_a[:],
                             start=True, stop=False)
            nc.tensor.matmul(ps_sum[:], ones_col[:], sum_b[:],
                             start=False, stop=True)
            # 1/x as exp(-ln(x)) on ScalarE: DVE reciprocal costs 3.4us even
            # on [1,512] (per-lane serial), ScalarE runs [1,512] in ~0.7us/op
            # and can emit float32r for the broadcast matmul directly
            lnd = st_pool.tile([1, QB], dt.float32, tag="lnd",
                               name=f"lnd{qb}")
            nc.scalar.activation(lnd[:], ps_sum[:], AF.Ln)
            rsum_r = st_pool.tile([1, QB], dt.float32r, tag="rsum",
                                  name=f"rsum{qb}")
            nc.scalar.activation(rsum_r[:], lnd[:], AF.Exp, scale=-1.0)
            filler(qb, 8)   # PE work while ScalarE runs ln/exp
            ps_b = ps_gen.tile([128, QB], dt.float32, tag="gen",
                               name=f"ps_b{qb}")
            nc.tensor.matmul(ps_b[:], ones_row[:], rsum_r[:],
                             start=True, stop=True)
            bcast = st_pool.tile([128, QB], dt.float32, tag="bcast",
                                 name=f"bcast{qb}")
            nc.scalar.copy(bcast[:], ps_b[:])
            filler(qb, 9)   # PE work while the broadcast is copied out
            st["bcast"] = bcast

        def _pv_group(qb, jk, pair, ps_w, start, stop):
            rhs = pair[:].rearrange("p (t q) -> p t q", t=2)
            for ht in range(HT):
                nc.tensor.matmul(ps_w[ht][:], vpair(jk, ht), rhs,
                                 start=start, stop=stop, perf_mode=PMODE)

        def weighted(qb):
            """weighted^T = PV * bcast + value^T; bf16 store + fp8 copy."""
            st = state[qb]
            wT_bf, wT_f8 = [], []
            for j in range(3):
                wT_bf.append(wt_pool.tile([128, 2 * QB], dt.bfloat16,
                                          tag=f"wTb{j}", name=f"wTb{j}_{qb}"))
                wT_f8.append(wt_pool.tile([128, 2 * QB], FP8,
                                          tag=f"wT8{j}", name=f"wT8{j}_{qb}"))
            for ht in range(HT):
                j, h2 = divmod(ht, 2)
                tmp = tmp_pool.tile([128, QB], dt.float32, tag="wtmp",
                                    name=f"wtmp_{qb}_{ht}")
                nc.vector.tensor_mul(tmp[:], st["ps_w"][ht][:], st["bcast"][:])
                nc.vector.tensor_add(
                    wT_bf[j][:, h2 * QB:(h2 + 1) * QB], tmp[:],
                    st["vT"][j][:, h2 * QB:(h2 + 1) * QB],
                )
                if h2 == 1:
                    if qb == 0:
                        nc.vector.tensor_copy(wT_f8[j][:], wT_bf[j][:])
                    else:
                        nc.scalar.copy(wT_f8[j][:], wT_bf[j][:])
            st["wT_bf"] = wT_bf
            st["wT_f8"] = wT_f8

        def mlp_h1_tile(qb, ot, rot=False):
            st = state[qb]
            if "h1" not in st:
                st["h1"] = [h1_pool.tile([128, 2 * QB], FP8, tag=f"h1_{j}",
                                         name=f"h1_{j}_{qb}")
                            for j in range(3)]
            if rot:
                ps = proj_ps(f"ps_h1_{qb}_{ot}")
            else:
                ps = ps_gen.tile([128, QB], dt.float32, tag="gen",
                                 name=f"ps_h1_{qb}_{ot}")
            for j in range(3):
                nc.tensor.matmul(
                    ps[:], wpair("wo1T", ot, j),
                    st["wT_f8"][j][:].rearrange("p (t q) -> p t q", t=2),
                    start=(j == 0), stop=(j == 2), perf_mode=PMODE,
                )
            j, h2 = divmod(ot, 2)
            nc.scalar.activation(
                st["h1"][j][:, h2 * QB:(h2 + 1) * QB], ps[:], AF.Relu,
                bias=biases["bo1"][:, ot: ot + 1], scale=1.0 / WSCALE,
            )

        def mlp_out_tile(qb, ot, rot=False):
            st = state[qb]
            if rot:
                ps = proj_ps(f"ps_o_{qb}_{ot}")
            else:
                ps = ps_gen.tile([128, QB], dt.float32, tag="gen",
                                 name=f"ps_o_{qb}_{ot}")
            for j in range(3):
                nc.tensor.matmul(
                    ps[:], wpair("wo2T", ot, j),
                    st["h1"][j][:].rearrange("p (t q) -> p t q", t=2),
                    start=(j == 0), stop=False, perf_mode=PMODE,
                )
            # residual rides the PE: PSUM += 64*I @ weighted^T (bf16, exact),
            # so the eviction is a single ScalarE op (scale 1/64 + bias) and
            # no DVE add sits on the output critical path
            j, h2 = divmod(ot, 2)
            nc.tensor.matmul(
                ps[:], id64_sb[:],
                st["wT_bf"][j][:, h2 * QB:(h2 + 1) * QB],
                start=False, stop=True,
            )
            o_sb = o_pool.tile([128, QB], dt.float32, tag="outT_blk",
                               name=f"outT_{qb}_{ot}")
            if qb == 0 or ot % 2 == 0:
                nc.scalar.activation(o_sb[:], ps[:], AF.Identity,
                                     bias=biases["bo2"][:, ot: ot + 1],
                                     scale=1.0 / WSCALE)
            else:
                nc.vector.tensor_scalar(o_sb[:], ps[:], 1.0 / WSCALE,
                                        biases["bo2"][:, ot: ot + 1],
                                        mybir.AluOpType.mult,
                                        mybir.AluOpType.add)
            eng = nc.sync if ot % 2 == 0 else nc.scalar
            eng.dma_start(
                outT_ext[:, (qb * HT + ot) * QB:(qb * HT + ot + 1) * QB],
                o_sb[:],
            )

        # ---- attention block 0: interleave Q-proj nb=1 tiles; slots 8/9
        # bracket the softmax normalization chain with PE work ----
        def fill0(qb, jk):
            if jk in (5, 6, 7):
                q_tile(1, qx1, jk - 5, gen_only=True)
            elif jk == 8:
                q_tile(1, qx1, 3, gen_only=True)
                q_tile(1, qx1, 4, gen_only=True)
                q_tile(1, qx1, 5, gen_only=True)

        attn_block(0, fill0)
        weighted(0)

        # ---- attention block 1: interleave MLP(0) h1 tiles ----
        def fill1(qb, jk):
            if jk in (5, 6, 7):
                mlp_h1_tile(0, jk - 5)
            elif jk == 8:
                mlp_h1_tile(0, 3)
                mlp_h1_tile(0, 4)
                mlp_h1_tile(0, 5)

        attn_block(1, fill1)
        # weighted(1) first: its DVE ops are the critical path into MLP(1);
        # mlp_out(0) and mlp_h1(1) alternate underneath it so the PE never
        # waits on a single eviction stream
        weighted(1)
        # the PE is in-order: emit all dependency-free mlp_out(0) tiles
        # before the first mlp_h1(1) tile (which waits for weighted(1)'s
        # first pair + fp8 cast). The MLP(1) tail rotates through the freed
        # PV PSUM banks so evictions never stall the 2-bank gen rotation.
        for ot in range(HT):
            mlp_out_tile(0, ot)
        for ot in range(HT):
            mlp_h1_tile(1, ot, rot=True)
        for ot in range(HT):
            mlp_out_tile(1, ot, rot=True)


# ---- host-side shard packing ----

def _tile_rows(a):
    """[T*128, N] -> [128, T*N]: partition-tiled T-layout, contiguous DMA."""
    t = a.shape[0] // 128
    return a.reshape(t, 128, a.shape[1]).transpose(1, 0, 2).reshape(128, -1)


def _tile_weight(w):
    """W^T [768h, 768o] -> [128, (ot, ht, 128)]: o-major packed lhsT tiles."""
    x = w.reshape(HT, 128, HT, 128)          # [ht, p, ot, o128]
    return x.transpose(1, 2, 0, 3).reshape(128, -1)


def _tile_rows_blocked(a, qb):
    """[768, NB*qb] -> [128, NB*(6*qb)]: per-block ht-major packing."""
    nb = a.shape[1] // qb
    x = a.reshape(HT, 128, nb, qb).transpose(1, 2, 0, 3)
    return x.reshape(128, -1)


def shard_inputs(query, key, value, Wq, bq, Wk, bk, Wo1, bo1, Wo2, bo2):
    """Full inputs -> per-core in_maps (host packing, fp8 cast, scale folds)."""
    scale = np.float32(1.0 / np.sqrt(np.float32(H)))

    def c8(x):
        return np.ascontiguousarray(
            np.clip(np.asarray(x, np.float32), -240, 240).astype(NP_FP8))

    def cb(x):
        return np.ascontiguousarray(np.asarray(x, np.float32).astype(NP_BF))

    def cf(x):
        return np.ascontiguousarray(x.astype(np.float32))

    shared = {
        "ident64": np.ascontiguousarray((np.eye(128, dtype=np.float32)
                                         * WSCALE).astype(NP_BF)),
        "wqT": c8(_tile_weight(Wq.T * (scale * QSCALE * WSCALE))),
        "wkT": c8(_tile_weight(Wk.T * WSCALE)),
        "wo1T": c8(_tile_weight(Wo1.T * WSCALE)),
        "wo2T": c8(_tile_weight(Wo2.T * WSCALE)),
        "biases": cf(np.concatenate([
            (bq * scale * QSCALE).reshape(HT, 128).T,
            bo1.reshape(HT, 128).T,
            bo2.reshape(HT, 128).T], axis=1)),
    }
    in_maps = []
    for core in range(N_CORES):
        b, half = divmod(core, 2)
        r0 = half * QCHUNK
        in_maps.append({
            "qT": c8(_tile_rows_blocked(query[b].T[:, r0: r0 + QCHUNK], QB)),
            "kT": c8(_tile_rows_blocked(key[b].T, QB)),
            "v": c8(_tile_rows(value[b])),
            "vT": cf(_tile_rows_blocked(value[b].T[:, r0: r0 + QCHUNK], QB)),
            **shared,
        })
    return in_maps


def gather_outputs(results):
    """Per-core outT [128, NQB*HT*QB] -> full [B, S, H]."""
    out = np.empty((B, S, H), dtype=np.float32)
    for core in range(N_CORES):
        b, half = divmod(core, 2)
        r0 = half * QCHUNK
        buf = results[core]["outT"].reshape(128, NQB, HT, QB)
        # out[q0+qb*QB+n, ot*128+p] = buf[p, qb, ot, n]
        out[b, r0: r0 + QCHUNK] = (
            buf.transpose(1, 3, 2, 0).reshape(QCHUNK, H)
        )
    return out


def run(inputs, trace=False):
    nc = build_kernel()
    in_maps = shard_inputs(**{k: np.asarray(v) for k, v in inputs.items()})
    res = run_bass_kernel_spmd(nc, in_maps, list(range(N_CORES)), trace=trace)
    return gather_outputs(res.results), res


def _split_multi_waits(nc):
    """Workaround for this container's walrus rejecting instructions that
    carry more than one semaphore wait ("Too many sync wait commands"):
    hoist N-1 waits onto fresh single-wait same-engine InstNoOp instructions
    inserted immediately before the instruction. Engine streams execute the
    block's per-engine subsequence in order, so blocking on the nops first is
    semantically identical to one multi-wait instruction."""
    for f in nc.m.functions:
        for bb in f.blocks:
            insts = list(bb.instructions)
            out = []
            changed = False
            for inst in insts:
                si = inst.sync_info
                waits = list(si.on_wait) if si is not None and si.on_wait else []
                if len(waits) > 1:
                    changed = True
                    for w in waits[:-1]:
                        nop = mybir.InstNoOp(
                            name=nc.get_next_instruction_name(), ins=[], outs=[]
                        )
                        nop.engine = inst.engine
                        nop.sync_info = mybir.SyncInfo(on_wait=[w], on_update=[])
                        out.append(nop)
                    si.on_wait = waits[-1:]
                    inst.sync_info = si
                out.append(inst)
            if changed:
                bb.instructions = out


def kernel(**inputs):
    """Entry point: full (unsharded) numpy inputs -> full [B, S, H] output."""
    out, _ = run(inputs, trace=False)
    return out


# revision 33
# speedup vs baseline: 1.1223x; 1.0024x over previous
"""Distributed single-head attention + MLP block for 8 TRN2 NeuronCores.

Reference computation (per batch b):
  Q = query @ Wq^T + bq ; K = key @ Wk^T + bk
  scores = Q @ K^T / sqrt(H) ; attn = softmax(scores)
  weighted = attn @ value + value
  h1 = relu(weighted @ Wo1^T + bo1)
  out = h1 @ Wo2^T + bo2 + weighted

Sharding: B=4 batches x 2 query-row halves = 8 shards. Each core gets its
1024 query rows plus the full 2048 keys/values of its batch; attention is
dense (non-causal) so no inter-core communication is needed.

Layout: everything on-device is feature-on-partitions ("T-layout",
X^T[f, tok]) so all matmul contractions line up with zero on-device
transposes; the host pre-packs every shard into the exact [128, free]
SBUF tiling the kernel consumes. All matmuls run fp8e4 DoubleRow (2x).

Numerics tricks (all folded on host / into activation scale operands):
  - softmax shift-invariance: K bias bk drops entirely (k-independent
    terms cancel; bq is kept on the Q side).
  - Q is stored as 32x true-Q and K as true-K so both live in fp8's
    normal range (raw Q*scale has std ~0.02 - subnormal); the scores
    PSUM is then 32x the true logits and the exp activation applies
    scale=1/32.
  - fp8 weights are stored 64x and unscaled via activation scale=1/64
    (uniform(-0.036, 0.036) weights would otherwise be ~half subnormal).
  - softmax needs no max-subtraction: logits have std ~1/3.

Schedule (PE never idles between phases):
  K-proj (24 tiles over an 8-bank PSUM rotation, DVE evictions)
  Q-proj nb=0, then attention block 0 with Q-proj nb=1 tiles interleaved
  attention block 1 with MLP(0) tiles interleaved, then MLP(1).
Attention merges scores and PV per k-tile pair: 3 score MMs -> exp on
ScalarE -> 6 PV MMs one pair behind, so PSUM-evict latency is always
covered by other matmuls. The softmax denominator accumulates on DVE
(even k-tiles) and GpSimd (odd k-tiles); the rowsum matmul sums both,
reciprocal runs on [1,512] only, and the PE broadcasts it back.
Bulk prefetches (v, vT, MLP weights) issue on the Scalar HWDGE queue
during the K-proj phase (ScalarE is idle there); everything else on Sync.
"""

import contextlib

import numpy as np
import ml_dtypes

import concourse.bass as bass
import concourse.mybir as mybir
import concourse.tile as tile
from concourse.bass_utils import run_bass_kernel_spmd

dt = mybir.dt
AF = mybir.ActivationFunctionType

H = 768          # model dim
B = 4            # batch
S = 2048         # sequence length
N_CORES = 8
QCHUNK = S * B // N_CORES        # 1024 query rows per core
HT = H // 128                    # 6 feature partition-tiles
KTILES = S // 128                # 16 key partition-tiles
QB = 512                         # q-block width (= PSUM bank, fp32)
NQB = QCHUNK // QB               # 2 q-blocks per core

FP8 = dt.float8e4
NP_FP8 = dt.np(FP8)
NP_BF = ml_dtypes.bfloat16
PMODE = mybir.MatmulPerfMode.DoubleRow

WSCALE = 64.0      # fp8 weight pre-scale (host) -> activation scale 1/64
QSCALE = 32.0      # stored Q = 32x true Q -> exp scale 1/32


def build_kernel():
    nc = bass.Bass()

    qT_ext = nc.declare_dram_parameter("qT", [128, NQB * HT * QB], FP8, isOutput=False)
    kT_ext = nc.declare_dram_parameter("kT", [128, (S // QB) * HT * QB], FP8, isOutput=False)
    v_ext = nc.declare_dram_parameter("v", [128, KTILES * H], FP8, isOutput=False)
    vT_ext = nc.declare_dram_parameter("vT", [128, NQB * HT * QB], dt.float32, isOutput=False)
    w_ext = {
        name: nc.declare_dram_parameter(name, [128, HT * H], FP8, isOutput=False)
        for name in ("wqT", "wkT", "wo1T", "wo2T")
    }
    b_ext = nc.declare_dram_parameter("biases", [128, 3 * HT], dt.float32,
                                      isOutput=False)
    id64_ext = nc.declare_dram_parameter("ident64", [128, 128], dt.bfloat16,
                                         isOutput=False)
    outT_ext = nc.declare_dram_parameter(
        "outT", [128, NQB * HT * QB], dt.float32, isOutput=True
    )

    with tile.TileContext(nc) as tc, nc.allow_low_precision(
        reason="fp8 matmul path is intentional; rel-err budget is 2e-2"
    ):
        _body(nc, tc, qT_ext, kT_ext, v_ext, vT_ext, w_ext, b_ext, id64_ext,
              outT_ext)

    _split_multi_waits(nc)
    return nc


def _body(nc, tc, qT_ext, kT_ext, v_ext, vT_ext, w_ext, b_ext, id64_ext,
          outT_ext):
    with contextlib.ExitStack() as ctx:
        const_pool = ctx.enter_context(tc.tile_pool(name="const", bufs=1))
        w_pool = ctx.enter_context(tc.tile_pool(name="w", bufs=1))
        act_pool = ctx.enter_context(tc.tile_pool(name="act", bufs=1))
        in_pool = ctx.enter_context(tc.tile_pool(name="inp", bufs=2))
        vt_pool = ctx.enter_context(tc.tile_pool(name="vt", bufs=2))
        wt_pool = ctx.enter_context(tc.tile_pool(name="wt", bufs=2))
        h1_pool = ctx.enter_context(tc.tile_pool(name="h1", bufs=2))
        st_pool = ctx.enter_context(tc.tile_pool(name="st", bufs=2))
        tmp_pool = ctx.enter_context(tc.tile_pool(name="tmp", bufs=4))
        o_pool = ctx.enter_context(tc.tile_pool(name="o", bufs=6))
        exp_pool = ctx.enter_context(tc.tile_pool(name="exps", bufs=6))
        # PSUM: 6 PV accumulators + 2 general banks = 8 banks exactly.
        ps_pool = ctx.enter_context(tc.tile_pool(name="ps", bufs=1, space="PSUM"))
        ps_gen = ctx.enter_context(tc.tile_pool(name="ps_gen", bufs=2, space="PSUM"))

        # ---- weight / bias loading helpers ----
        w_sb = {}

        def load_weight_chunk(name, j, eng=None):
            ts = w_sb.get(name)
            if ts is None:
                ts = [w_pool.tile([128, HT * 128], FP8, tag=f"{name}{g}",
                                  name=f"w_{name}{g}") for g in range(HT)]
                w_sb[name] = ts
            step = HT * 128
            (eng or nc.sync).dma_start(ts[j][:],
                                       w_ext[name][:, j * step:(j + 1) * step])

        def wpair(name, ot, j):
            """lhsT [128, 2, 128]: out-tile ot, contraction h-tile pair (2j, 2j+1)."""
            c0 = 2 * j * 128
            return (w_sb[name][ot][:, c0: c0 + 256]
                    .rearrange("p (t m) -> p t m", t=2))

        # ---- stage-1 DMAs in first-use order (Sync queue) ----
        def load_x_chunks(ext, nb, tagbase):
            """One projection input block as 3 ht-pair tiles [128, 2*QB]."""
            tiles = []
            for j in range(3):
                t = in_pool.tile([128, 2 * QB], FP8, tag=f"{tagbase}{j}",
                                 name=f"x_{tagbase}{j}_{nb}")
                c0 = nb * HT * QB + j * 2 * QB
                nc.sync.dma_start(t[:], ext[:, c0: c0 + 2 * QB])
                tiles.append(t)
            return tiles

        load_weight_chunk("wkT", 0)
        kx0 = []
        for j in range(3):
            t = in_pool.tile([128, 2 * QB], FP8, tag=f"kx{j}", name=f"x_kx{j}_0")
            nc.sync.dma_start(t[:], kT_ext[:, j * 2 * QB:(j + 1) * 2 * QB])
            kx0.append(t)
        for j in range(1, HT):
            load_weight_chunk("wkT", j, eng=nc.scalar)

        bias_sb = const_pool.tile([128, 3 * HT], dt.float32, tag="biases")
        nc.sync.dma_start(bias_sb[:], b_ext[:])
        id64_sb = const_pool.tile([128, 128], dt.bfloat16, tag="ident64")
        nc.scalar.dma_start(id64_sb[:], id64_ext[:])
        biases = {name: bias_sb[:, i * HT:(i + 1) * HT]
                  for i, name in enumerate(("bq", "bo1", "bo2"))}

        # ---- activation tiles ----
        KT = [act_pool.tile([128, HT * QB], FP8, tag=f"KT{nb}",
                            name=f"KT{nb}") for nb in range(4)]
        KT3 = [t[:].rearrange("p (t k) -> p t k", t=HT) for t in KT]
        QT = [[act_pool.tile([128, 2 * QB], FP8, tag=f"QT{qb}_{j}",
                             name=f"QT{qb}_{j}") for j in range(3)]
              for qb in range(NQB)]

        ones_f32 = const_pool.tile([128, 128], dt.float32, tag="ones_f32")
        nc.vector.memset(ones_f32[:], 1.0)
        ones_row = const_pool.tile([1, 128], dt.float32r, tag="ones_row")
        nc.vector.tensor_copy(ones_row[:], ones_f32[0:1, :])
        ones_col = const_pool.tile([128, 1], dt.float32r, tag="ones_col")
        nc.vector.tensor_copy(ones_col[:], ones_f32[:, 0:1])

        # 8-bank PSUM rotation for the projection phase (PV banks are free).
        _rot = {"i": 0}

        def proj_ps(nm):
            i = _rot["i"]
            _rot["i"] += 1
            if i % 8 < 2:
                return ps_gen.tile([128, QB], dt.float32, tag="gen", name=nm)
            return ps_pool.tile([128, QB], dt.float32, tag=f"ps_w{i % 8 - 2}",
                                name=nm)

        def proj_tile(wname, xt, ot, ps):
            """3 DoubleRow matmuls: one [128out, QB] projection tile."""
            for j in range(3):
                nc.tensor.matmul(
                    ps[:], wpair(wname, ot, j),
                    xt[j][:].rearrange("p (t q) -> p t q", t=2),
                    start=(j == 0), stop=(j == 2), perf_mode=PMODE,
                )

        # ---- K projection: 24 tiles, DVE evictions (no bias - bk drops) ----
        k_tile_fns = []

        def k_tile(nb, xt, ot):
            ps = ps_gen.tile([128, QB], dt.float32, tag="gen",
                             name=f"ps_k3_{ot}")
            proj_tile("wkT", xt, ot, ps)
            nc.vector.tensor_scalar_mul(KT[nb][:, ot * QB:(ot + 1) * QB],
                                        ps[:], 1.0 / WSCALE)

        cur = kx0
        for nb in range(4):
            nxt = load_x_chunks(kT_ext, nb + 1, "kx") if nb < 3 else None
            if nb == 1:
                v_blks = []
                for c in range(4):
                    t = act_pool.tile([128, 4 * H], FP8, tag=f"v_in{c}",
                                      name=f"v_in{c}")
                    nc.scalar.dma_start(t[:], v_ext[:, c * 4 * H:(c + 1) * 4 * H])
                    v_blks.append(t)
            elif nb == 2:
                pass
            for ot in range(HT):
                ps = proj_ps(f"ps_k_{nb}_{ot}")
                proj_tile("wkT", cur, ot, ps)
                dst = KT[nb][:, ot * QB:(ot + 1) * QB]
                # evictions on DVE, except nb3 on ScalarE (by then its
                # descriptor-generation burst is done and it idles, while
                # DVE alone falls ~1.4us behind the rotation)
                if nb == 3:
                    nc.scalar.activation(dst, ps[:], AF.Identity,
                                         scale=1.0 / WSCALE)
                else:
                    nc.vector.tensor_scalar_mul(dst, ps[:], 1.0 / WSCALE)
            cur = nxt

        def vpair(jk, ht):
            """lhsT [128, 2, 128]: k-tile pair (2jk, 2jk+1), h-tile ht."""
            t = v_blks[jk // 2]
            j2 = (jk % 2) * 2
            return (t[:].rearrange("p (t h) -> p t h", t=4)
                    [:, j2: j2 + 2, ht * 128:(ht + 1) * 128])

        # ---- Q projection nb=0 (ScalarE evictions apply bias + 1/64) ----
        for j in range(HT):
            load_weight_chunk("wqT", j)
        qx0 = load_x_chunks(qT_ext, 0, "qx")
        qx1 = load_x_chunks(qT_ext, 1, "qx")
        for j in range(HT):
            load_weight_chunk("wo1T", j)
        for j in range(HT):
            load_weight_chunk("wo2T", j)

        def q_tile(qb, xt, ot, gen_only=False):
            # interleaved tiles (inside attention) must not touch the live
            # PV accumulator banks - gen rotation only
            if gen_only:
                ps = ps_gen.tile([128, QB], dt.float32, tag="gen",
                                 name=f"ps_q_{qb}_{ot}")
            else:
                ps = proj_ps(f"ps_q_{qb}_{ot}")
            proj_tile("wqT", xt, ot, ps)
            dst = QT[qb][ot // 2][:, (ot % 2) * QB:(ot % 2 + 1) * QB]
            if gen_only or ot < 2:
                # inside attention ScalarE is busy with exps; and ot0/ot1
                # hold the gen banks the first scores need - evict on DVE
                nc.vector.tensor_scalar(dst, ps[:], 1.0 / WSCALE,
                                        biases["bq"][:, ot: ot + 1],
                                        mybir.AluOpType.mult,
                                        mybir.AluOpType.add)
            else:
                nc.scalar.activation(dst, ps[:], AF.Identity,
                                     bias=biases["bq"][:, ot: ot + 1],
                                     scale=1.0 / WSCALE)

        # ---- vT (bf16 residual) as ht-pair tiles ----
        def load_vt(qb, eng):
            tiles = []
            for j in range(3):
                t = vt_pool.tile([128, 2 * QB], dt.float32, tag=f"vT{j}",
                                 name=f"vT{j}_{qb}")
                c0 = qb * HT * QB + j * 2 * QB
                eng.dma_start(t[:], vT_ext[:, c0: c0 + 2 * QB])
                tiles.append(t)
            return tiles

        state = {}
        state[0] = {"vT": load_vt(0, nc.sync)}
        for ot in range(HT):
            q_tile(0, qx0, ot)

        def attn_block(qb, filler):
            """Merged scores+PV for q-block qb; filler(slot) emits interleaved
            PE work (proj/MLP tiles) - called with slot index 0..7 per jk."""
            st = state.setdefault(qb, {})
            if "vT" not in st:
                st["vT"] = load_vt(qb, nc.sync)
            sum_a = st_pool.tile([128, QB], dt.float32r, tag="sum_a",
                                 name=f"sum_a{qb}")
            sum_b = st_pool.tile([128, QB], dt.float32r, tag="sum_b",
                                 name=f"sum_b{qb}")
            ps_w = [ps_pool.tile([128, QB], dt.float32, tag=f"ps_w{ht}",
                                 name=f"ps_w{ht}_{qb}")
                    for ht in range(HT)]
            exp_pairs = []
            for jk in range(KTILES // 2):
                pair = exp_pool.tile([128, 2 * QB], FP8, tag="expS",
                                     name=f"expS_{qb}_{jk}")
                exp_pairs.append(pair)
                for t2 in range(2):
                    kt = 2 * jk + t2
                    ps_s = ps_gen.tile([128, QB], dt.float32, tag="gen",
                                       name=f"ps_s_{qb}_{kt}")
                    for jo in range(3):
                        nc.tensor.matmul(
                            ps_s[:],
                            KT3[kt // 4][:, 2 * jo: 2 * jo + 2,
                                         (kt % 4) * 128:(kt % 4 + 1) * 128],
                            QT[qb][jo][:].rearrange("p (t q) -> p t q", t=2),
                            start=(jo == 0), stop=(jo == 2), perf_mode=PMODE,
                        )
                    half = pair[:, t2 * QB:(t2 + 1) * QB]
                    nc.scalar.activation(half, ps_s[:], AF.Exp,
                                         scale=1.0 / QSCALE)
                    # two DVE accumulators (GpSimd fp8 2-input ops measured
                    # far too slow); two chains halve the serial dependency
                    if kt == 0:
                        nc.vector.tensor_copy(sum_b[:], half)
                    elif kt == 1:
                        nc.vector.tensor_copy(sum_a[:], half)
                    elif kt % 2 == 0:
                        nc.vector.tensor_add(sum_b[:], sum_b[:], half)
                    else:
                        nc.vector.tensor_add(sum_a[:], sum_a[:], half)
                filler(qb, jk)
                if jk >= 1:
                    _pv_group(qb, jk - 1, exp_pairs[jk - 1], ps_w,
                              start=(jk == 1), stop=False)
            _pv_group(qb, 7, exp_pairs[7], ps_w, start=False, stop=True)
            st["ps_w"] = ps_w

            # rowsum (both accumulators) -> 1/x on [1,512] -> PE broadcast
            ps_sum = ps_gen.tile([1, QB], dt.float32, tag="gen",
                                 name=f"ps_sum{qb}")
            nc.tensor.matmul(ps_sum[:], ones_col[:], sum_a[:],
                             start=True, stop=False)
            nc.tensor.matmul(ps_sum[:], ones_col[:], sum_b[:],
                             start=False, stop=True)
            # 1/x as exp(-ln(x)) on ScalarE: DVE reciprocal costs 3.4us even
            # on [1,512] (per-lane serial), ScalarE runs [1,512] in ~0.7us/op
            # and can emit float32r for the broadcast matmul directly
            lnd = st_pool.tile([1, QB], dt.float32, tag="lnd",
                               name=f"lnd{qb}")
            nc.scalar.activation(lnd[:], ps_sum[:], AF.Ln)
            rsum_r = st_pool.tile([1, QB], dt.float32r, tag="rsum",
                                  name=f"rsum{qb}")
            nc.scalar.activation(rsum_r[:], lnd[:], AF.Exp, scale=-1.0)
            filler(qb, 8)   # PE work while ScalarE runs ln/exp
            ps_b = ps_gen.tile([128, QB], dt.float32, tag="gen",
                               name=f"ps_b{qb}")
            nc.tensor.matmul(ps_b[:], ones_row[:], rsum_r[:],
                             start=True, stop=True)
            bcast = st_pool.tile([128, QB], dt.float32, tag="bcast",
                                 name=f"bcast{qb}")
            nc.scalar.copy(bcast[:], ps_b[:])
            filler(qb, 9)   # PE work while the broadcast is copied out
            st["bcast"] = bcast

        def _pv_group(qb, jk, pair, ps_w, start, stop):
            rhs = pair[:].rearrange("p (t q) -> p t q", t=2)
            for ht in range(HT):
                nc.tensor.matmul(ps_w[ht][:], vpair(jk, ht), rhs,
                                 start=start, stop=stop, perf_mode=PMODE)

        def weighted(qb):
            """weighted^T = PV * bcast + value^T; bf16 store + fp8 copy."""
            st = state[qb]
            wT_bf, wT_f8 = [], []
            for j in range(3):
                wT_bf.append(wt_pool.tile([128, 2 * QB], dt.bfloat16,
                                          tag=f"wTb{j}", name=f"wTb{j}_{qb}"))
                wT_f8.append(wt_pool.tile([128, 2 * QB], FP8,
                                          tag=f"wT8{j}", name=f"wT8{j}_{qb}"))
            for ht in range(HT):
                j, h2 = divmod(ht, 2)
                tmp = tmp_pool.tile([128, QB], dt.float32, tag="wtmp",
                                    name=f"wtmp_{qb}_{ht}")
                nc.vector.tensor_mul(tmp[:], st["ps_w"][ht][:], st["bcast"][:])
                nc.vector.tensor_add(
                    wT_bf[j][:, h2 * QB:(h2 + 1) * QB], tmp[:],
                    st["vT"][j][:, h2 * QB:(h2 + 1) * QB],
                )
                if h2 == 1:
                    if qb == 0:
                        nc.vector.tensor_copy(wT_f8[j][:], wT_bf[j][:])
                    else:
                        nc.scalar.copy(wT_f8[j][:], wT_bf[j][:])
            st["wT_bf"] = wT_bf
            st["wT_f8"] = wT_f8

        def mlp_h1_tile(qb, ot, rot=False):
            st = state[qb]
            if "h1" not in st:
                st["h1"] = [h1_pool.tile([128, 2 * QB], FP8, tag=f"h1_{j}",
                                         name=f"h1_{j}_{qb}")
                            for j in range(3)]
            if rot:
                ps = proj_ps(f"ps_h1_{qb}_{ot}")
            else:
                ps = ps_gen.tile([128, QB], dt.float32, tag="gen",
                                 name=f"ps_h1_{qb}_{ot}")
            for j in range(3):
                nc.tensor.matmul(
                    ps[:], wpair("wo1T", ot, j),
                    st["wT_f8"][j][:].rearrange("p (t q) -> p t q", t=2),
                    start=(j == 0), stop=(j == 2), perf_mode=PMODE,
                )
            j, h2 = divmod(ot, 2)
            nc.scalar.activation(
                st["h1"][j][:, h2 * QB:(h2 + 1) * QB], ps[:], AF.Relu,
                bias=biases["bo1"][:, ot: ot + 1], scale=1.0 / WSCALE,
            )

        def mlp_out_tile(qb, ot, rot=False):
            st = state[qb]
            if rot:
                ps = proj_ps(f"ps_o_{qb}_{ot}")
            else:
                ps = ps_gen.tile([128, QB], dt.float32, tag="gen",
                                 name=f"ps_o_{qb}_{ot}")
            for j in range(3):
                nc.tensor.matmul(
                    ps[:], wpair("wo2T", ot, j),
                    st["h1"][j][:].rearrange("p (t q) -> p t q", t=2),
                    start=(j == 0), stop=False, perf_mode=PMODE,
                )
            # residual rides the PE: PSUM += 64*I @ weighted^T (bf16, exact),
            # so the eviction is a single ScalarE op (scale 1/64 + bias) and
            # no DVE add sits on the output critical path
            j, h2 = divmod(ot, 2)
            nc.tensor.matmul(
                ps[:], id64_sb[:],
                st["wT_bf"][j][:, h2 * QB:(h2 + 1) * QB],
                start=False, stop=True,
            )
            o_sb = o_pool.tile([128, QB], dt.float32, tag="outT_blk",
                               name=f"outT_{qb}_{ot}")
            if qb == 0 or ot % 2 == 0:
                nc.scalar.activation(o_sb[:], ps[:], AF.Identity,
                                     bias=biases["bo2"][:, ot: ot + 1],
                                     scale=1.0 / WSCALE)
            else:
                nc.vector.tensor_scalar(o_sb[:], ps[:], 1.0 / WSCALE,
                                        biases["bo2"][:, ot: ot + 1],
                                        mybir.AluOpType.mult,
                                        mybir.AluOpType.add)
            eng = nc.sync if ot % 2 == 0 else nc.scalar
            eng.dma_start(
                outT_ext[:, (qb * HT + ot) * QB:(qb * HT + ot + 1) * QB],
                o_sb[:],
            )

        # ---- attention block 0: interleave Q-proj nb=1 tiles; slots 8/9
        # bracket the softmax normalization chain with PE work ----
        def fill0(qb, jk):
            if jk in (5, 6, 7):
                q_tile(1, qx1, jk - 5, gen_only=True)
            elif jk == 8:
                q_tile(1, qx1, 3, gen_only=True)
                q_tile(1, qx1, 4, gen_only=True)
                q_tile(1, qx1, 5, gen_only=True)

        attn_block(0, fill0)
        weighted(0)

        # ---- attention block 1: interleave MLP(0) h1 tiles ----
        def fill1(qb, jk):
            if jk in (5, 6, 7):
                mlp_h1_tile(0, jk - 5)
            elif jk == 8:
                mlp_h1_tile(0, 3)
                mlp_h1_tile(0, 4)
                mlp_h1_tile(0, 5)

        attn_block(1, fill1)
        # weighted(1) first: its DVE ops are the critical path into MLP(1);
        # mlp_out(0) and mlp_h1(1) alternate underneath it so the PE never
        # waits on a single eviction stream
        weighted(1)
        # the PE is in-order: emit all dependency-free mlp_out(0) tiles
        # before the first mlp_h1(1) tile (which waits for weighted(1)'s
        # first pair + fp8 cast). The MLP(1) tail rotates through the freed
        # PV PSUM banks so evictions never stall the 2-bank gen rotation.
        for ot in range(HT):
            mlp_out_tile(0, ot)
        for ot in range(HT):
            mlp_h1_tile(1, ot, rot=True)
        for ot in range(HT):
            mlp_out_tile(1, ot, rot=True)


# ---- host-side shard packing ----

def _tile_rows(a):
    """[T*128, N] -> [128, T*N]: partition-tiled T-layout, contiguous DMA."""
    t = a.shape[0] // 128
    return a.reshape(t, 128, a.shape[1]).transpose(1, 0, 2).reshape(128, -1)


def _tile_weight(w):
    """W^T [768h, 768o] -> [128, (ot, ht, 128)]: o-major packed lhsT tiles."""
    x = w.reshape(HT, 128, HT, 128)          # [ht, p, ot, o128]
    return x.transpose(1, 2, 0, 3).reshape(128, -1)


def _tile_rows_blocked(a, qb):
    """[768, NB*qb] -> [128, NB*(6*qb)]: per-block ht-major packing."""
    nb = a.shape[1] // qb
    x = a.reshape(HT, 128, nb, qb).transpose(1, 2, 0, 3)
    return x.reshape(128, -1)


def shard_inputs(query, key, value, Wq, bq, Wk, bk, Wo1, bo1, Wo2, bo2):
    """Full inputs -> per-core in_maps (host packing, fp8 cast, scale folds)."""
    scale = np.float32(1.0 / np.sqrt(np.float32(H)))

    def c8(x):
        return np.ascontiguousarray(
            np.clip(np.asarray(x, np.float32), -240, 240).astype(NP_FP8))

    def cb(x):
        return np.ascontiguousarray(np.asarray(x, np.float32).astype(NP_BF))

    def cf(x):
        return np.ascontiguousarray(x.astype(np.float32))

    shared = {
        "ident64": np.ascontiguousarray((np.eye(128, dtype=np.float32)
                                         * WSCALE).astype(NP_BF)),
        "wqT": c8(_tile_weight(Wq.T * (scale * QSCALE * WSCALE))),
        "wkT": c8(_tile_weight(Wk.T * WSCALE)),
        "wo1T": c8(_tile_weight(Wo1.T * WSCALE)),
        "wo2T": c8(_tile_weight(Wo2.T * WSCALE)),
        "biases": cf(np.concatenate([
            (bq * scale * QSCALE).reshape(HT, 128).T,
            bo1.reshape(HT, 128).T,
            bo2.reshape(HT, 128).T], axis=1)),
    }
    in_maps = []
    for core in range(N_CORES):
        b, half = divmod(core, 2)
        r0 = half * QCHUNK
        in_maps.append({
            "qT": c8(_tile_rows_blocked(query[b].T[:, r0: r0 + QCHUNK], QB)),
            "kT": c8(_tile_rows_blocked(key[b].T, QB)),
            "v": c8(_tile_rows(value[b])),
            "vT": cf(_tile_rows_blocked(value[b].T[:, r0: r0 + QCHUNK], QB)),
            **shared,
        })
    return in_maps


def gather_outputs(results):
    """Per-core outT [128, NQB*HT*QB] -> full [B, S, H]."""
    out = np.empty((B, S, H), dtype=np.float32)
    for core in range(N_CORES):
        b, half = divmod(core, 2)
        r0 = half * QCHUNK
        buf = results[core]["outT"].reshape(128, NQB, HT, QB)
        # out[q0+qb*QB+n, ot*128+p] = buf[p, qb, ot, n]
        out[b, r0: r0 + QCHUNK] = (
            buf.transpose(1, 3, 2, 0).reshape(QCHUNK, H)
        )
    return out


def run(inputs, trace=False):
    nc = build_kernel()
    in_maps = shard_inputs(**{k: np.asarray(v) for k, v in inputs.items()})
    res = run_bass_kernel_spmd(nc, in_maps, list(range(N_CORES)), trace=trace)
    return gather_outputs(res.results), res


def _split_multi_waits(nc):
    """Workaround for this container's walrus rejecting instructions that
    carry more than one semaphore wait ("Too many sync wait commands"):
    hoist N-1 waits onto fresh single-wait same-engine InstNoOp instructions
    inserted immediately before the instruction. Engine streams execute the
    block's per-engine subsequence in order, so blocking on the nops first is
    semantically identical to one multi-wait instruction."""
    for f in nc.m.functions:
        for bb in f.blocks:
            insts = list(bb.instructions)
            out = []
            changed = False
            for inst in insts:
                si = inst.sync_info
                waits = list(si.on_wait) if si is not None and si.on_wait else []
                if len(waits) > 1:
                    changed = True
                    for w in waits[:-1]:
                        nop = mybir.InstNoOp(
                            name=nc.get_next_instruction_name(), ins=[], outs=[]
                        )
                        nop.engine = inst.engine
                        nop.sync_info = mybir.SyncInfo(on_wait=[w], on_update=[])
                        out.append(nop)
                    si.on_wait = waits[-1:]
                    inst.sync_info = si
                out.append(inst)
            if changed:
                bb.instructions = out


def kernel(**inputs):
    """Entry point: full (unsharded) numpy inputs -> full [B, S, H] output."""
    out, _ = run(inputs, trace=False)
    return out
